# revision 14
# baseline (speedup 1.0000x reference)
"""Trainium2 Bass kernel for nn_DSQGAttentionQW (sparse offset attention).

Sharding: head-tensor-parallel attention (8 heads -> 8 cores) + AllToAll
re-shard to sequence-parallel for the output projection. Single NEFF launch.
"""
import math
import numpy as np

import concourse.bacc as bacc
import concourse.bass as bass
import concourse.tile as tile
import concourse.mybir as mybir
import concourse.masks as masks
from concourse.bass_utils import run_bass_kernel_spmd

# ---- problem constants (must match reference.py) ----
_DENSE_LOCAL_W = 32
_DYADIC = [48, 64, 96, 128, 192, 256, 384, 512, 768, 1024, 1536, 2048, 3072, 4096]
OFFSETS = np.array(
    sorted(set(range(0, _DENSE_LOCAL_W + 1)) | set(_DYADIC)), dtype=np.int32
)  # [47]
NUM_OFFSETS = len(OFFSETS)
H = 8
_LOG_MAX = math.log(1.0 + 4096.0)
_HEAD_OMEGAS = [0.0, 0.0, 1 * math.pi / _LOG_MAX, 1 * math.pi / _LOG_MAX,
                4 * math.pi / _LOG_MAX, 4 * math.pi / _LOG_MAX,
                6 * math.pi / _LOG_MAX, 6 * math.pi / _LOG_MAX]
_log_d = np.log(1.0 + OFFSETS.astype(np.float64))
DISP_COS_KERNEL = np.zeros((NUM_OFFSETS, H), dtype=np.float32)
for _h, _om in enumerate(_HEAD_OMEGAS):
    if _om > 0.0:
        DISP_COS_KERNEL[:, _h] = np.cos(_om * _log_d)

B, N, D = 1, 2048, 512
HD = D // H
NC = 8
NB = N // NC            # 256: per-core output row block
NT = N // 128           # 16 q-tiles of 128
# Effective k-tile depths m (delta in (128(m-1), 128m]) that can be causal for
# N=2048: depths 16/24/32 (delta >= 2048) are never valid.
R_DEPTHS = [0, 1, 2, 3, 4, 6, 8, 12]
NR = len(R_DEPTHS)

FP = mybir.dt.float32
FR = mybir.dt.float32r
BF = mybir.dt.bfloat16

_cache = {}


def _build_masks(eff_pb_h: np.ndarray) -> np.ndarray:
    """maskW[ri, kp, i] = exp(eff_pb[offset_idx(delta)]) if delta valid else 0,
    with delta = i - kp + 128*m for depth m = R_DEPTHS[ri]."""
    off_idx = {int(d): i for i, d in enumerate(OFFSETS)}
    kp = np.arange(128)[None, :, None]
    i = np.arange(128)[None, None, :]
    m = np.array(R_DEPTHS)[:, None, None]
    delta = i - kp + 128 * m  # [NR, 128, 128]
    w = np.zeros((NR, 128, 128), dtype=np.float32)
    for d, oi in off_idx.items():
        sel = delta == d
        if sel.any():
            w[sel] = math.exp(float(eff_pb_h[oi]))
    return w


def _r(ap):
    return ap.bitcast(FR)


def _build_module():
    nc = bacc.Bacc("TRN2", target_bir_lowering=False, debug=False, num_devices=NC)

    xT = nc.dram_tensor("xT", [D, N], FR, kind="ExternalInput").ap()
    wA = nc.dram_tensor("wA", [D, 128], FR, kind="ExternalInput").ap()   # [Wq|Wk]
    wB = nc.dram_tensor("wB", [D, 128], FR, kind="ExternalInput").ap()   # [Wv|Wg]
    bA = nc.dram_tensor("bA", [128], FP, kind="ExternalInput").ap()
    bB = nc.dram_tensor("bB", [128], FP, kind="ExternalInput").ap()
    maskW = nc.dram_tensor("maskW", [NR, 128, 128], FP, kind="ExternalInput").ap()
    woutS = nc.dram_tensor("woutS", [HD, H, D], FR, kind="ExternalInput").ap()
    bout = nc.dram_tensor("bout", [D], FR, kind="ExternalInput").ap()
    yout = nc.dram_tensor("y", [NB, D], FP, kind="ExternalOutput").ap()

    with tile.TileContext(nc) as tc:
        with (
            tc.tile_pool(name="singles", bufs=1) as S,
            tc.tile_pool(name="work", bufs=3) as W,
            tc.tile_pool(name="pk", bufs=2) as PK,
            tc.tile_pool(name="ps", bufs=2, space="PSUM") as PS,
            tc.tile_pool(name="ps3", bufs=2, space="PSUM") as PS3,
            tc.tile_pool(name="pso", bufs=1, space="PSUM") as PSO,
            tc.tile_pool(name="dram", bufs=1, space="DRAM") as DR,
        ):
            # ---------- constants / loads ----------
            ident = S.tile([128, 128], FP)
            masks.make_identity(nc, ident[:])
            ones_f = S.tile([1, 128], FP)
            nc.vector.memset(ones_f[:], 1.0)
            ones_r = S.tile([1, 128], FR)
            nc.vector.tensor_copy(ones_r[:], ones_f[:])

            xs = S.tile([128, 4, N], FR)
            xT_r = xT.rearrange("(ct p) n -> p ct n", p=128)
            for ct in range(4):
                nc.sync.dma_start(out=xs[:, ct, :], in_=xT_r[:, ct, :])

            wAs = S.tile([128, 4, 128], FR)
            nc.sync.dma_start(out=wAs[:], in_=wA.rearrange("(ct p) o -> p ct o", p=128))
            wBs = S.tile([128, 4, 128], FR)
            nc.sync.dma_start(out=wBs[:], in_=wB.rearrange("(ct p) o -> p ct o", p=128))
            bAs = S.tile([128, 1], FP)
            nc.sync.dma_start(out=bAs[:], in_=bA[:, None])
            bBs = S.tile([128, 1], FP)
            nc.sync.dma_start(out=bBs[:], in_=bB[:, None])
            mws = S.tile([128, NR, 128], FP)
            nc.sync.dma_start(out=mws[:], in_=maskW.rearrange("r kp i -> kp r i"))
            wos = S.tile([HD, H, D], FR)
            nc.sync.dma_start(out=wos[:], in_=woutS[:])
            bos = S.tile([1, D], FR)
            nc.sync.dma_start(out=bos[:], in_=bout[None, :])

            # ---------- MM-A: qT / kT / vT / gT (fp32r, N=512) ----------
            qT = S.tile([64, N], FR)       # pre-scaled by 1/sqrt(HD)
            kT = S.tile([64, N], FR)
            vT = S.tile([64, N], FP)       # if_gain folded
            gT = S.tile([HD + 1, N], FP)   # sigmoid gate; row 64 = 1.0 (denom)
            nc.vector.memset(gT[HD:HD + 1, :], 1.0)
            for nch in range(4):
                nsl = slice(512 * nch, 512 * (nch + 1))
                psA = PS.tile([128, 512], FP, tag="mma")
                psB = PS.tile([128, 512], FP, tag="mma")
                for ct in range(4):
                    nc.tensor.matmul(psA[:], wAs[:, ct, :], xs[:, ct, nsl],
                                     start=(ct == 0), stop=(ct == 3))
                for ct in range(4):
                    nc.tensor.matmul(psB[:], wBs[:, ct, :], xs[:, ct, nsl],
                                     start=(ct == 0), stop=(ct == 3))
                # biased copies: q,k,v on DVE; gate sigmoid on ACT
                nc.vector.tensor_scalar_add(qT[:, nsl], psA[0:64, :], bAs[0:64])
                nc.vector.tensor_scalar_add(kT[:, nsl], psA[64:128, :], bAs[64:128])
                nc.vector.tensor_scalar_add(vT[:, nsl], psB[0:64, :], bBs[0:64])
                nc.scalar.activation(gT[0:HD, nsl], psB[64:128, :],
                                     mybir.ActivationFunctionType.Sigmoid,
                                     bias=bBs[64:128], scale=1.0)

            # ---------- V natural (fp32r) + ones column ----------
            Vn = S.tile([128, NT, HD + 1], FR)
            onesNT = S.tile([128, NT, 1], FP)
            nc.vector.memset(onesNT[:], 1.0)
            nc.vector.tensor_copy(Vn[:, :, HD:HD + 1], onesNT[:])
            for t in range(NT):
                psT = PS3.tile([128, 64], FP, tag="s")
                nc.tensor.transpose(psT[:], vT[:, 128 * t:128 * (t + 1)],
                                    ident[0:64, 0:64])
                nc.scalar.copy(Vn[:, t, 0:HD], psT[:])

            # ---------- attention (k-tile-major, fp32r) ----------
            # out2_all: [65, N] PSUM accumulator (4 banks). Zero-init via K=1
            # start=True matmuls so subsequent MMs accumulate via has_written.
            out2_all = PSO.tile([HD + 1, N], FP)
            zrow_f = S.tile([1, 512], FP)
            nc.vector.memset(zrow_f[:], 0.0)
            zrow = S.tile([1, 512], FR)
            nc.vector.tensor_copy(zrow[:], zrow_f[:])
            z65 = S.tile([1, HD + 1], FR)
            nc.vector.tensor_copy(z65[:], zrow_f[:, 0:HD + 1])
            for c in range(4):
                nc.tensor.matmul(out2_all[:, 512 * c:512 * (c + 1)], z65[:],
                                 zrow[:], start=True, stop=False)

            zX = S.tile([HD + 1, N], FP)  # gated attn out^T + denom row

            for tk in range(NT):
                ksl = slice(128 * tk, 128 * (tk + 1))
                groups = []
                nq_near = min(3, NT - tk)
                groups.append(([0, 1, 2][:nq_near], tk, nq_near))
                nq34 = max(0, min(2, NT - tk - 3))
                if nq34:
                    groups.append(([3, 4][:nq34], tk + 3, nq34))
                if tk + 8 < NT:
                    groups.append(([6, None, 8], tk + 6, 3))
                elif tk + 6 < NT:
                    groups.append(([6], tk + 6, 1))
                if tk + 12 < NT:
                    groups.append(([12], tk + 12, 1))
                Pk = PK.tile([128, NR, 128], FR, tag="Pk")
                for ms, q_lo, nq in groups:
                    qsl = slice(128 * q_lo, 128 * (q_lo + nq))
                    psS = PS3.tile([128, 384], FP, tag="s")
                    nc.tensor.matmul(psS[:, 0:128 * nq], kT[:, ksl],
                                     qT[:, qsl], start=True, stop=True)
                    expS = W.tile([128, 384], FP, tag="expS")
                    nc.scalar.activation(expS[:, 0:128 * nq], psS[:, 0:128 * nq],
                                         mybir.ActivationFunctionType.Exp)
                    ris = [R_DEPTHS.index(m) for m in ms if m is not None]
                    if ms == [6, None, 8]:
                        src = bass.AP(tensor=expS.tensor, offset=expS[:].offset,
                                      ap=[expS[:].ap[0], [256, 2], [1, 128]])
                        nc.vector.tensor_mul(Pk[:, ris[0]:ris[0] + 2, :],
                                             src, mws[:, ris[0]:ris[0] + 2, :])
                    else:
                        nc.vector.tensor_mul(
                            Pk[:, ris[0]:ris[0] + len(ris), :],
                            expS[:, 0:128 * len(ris)],
                            mws[:, ris[0]:ris[0] + len(ris), :])

                # ---- MM2 for k-tile tk: accumulate into out2_all q-columns ----
                nq1 = min(4, NT - tk)
                nc.tensor.matmul(
                    out2_all[:, 128 * tk:128 * (tk + nq1)],
                    Vn[:, tk, :], Pk[:, 0:nq1, :],
                    start=False, stop=(tk == NT - 1), skip_group_check=True)
                for ri, m in ((4, 4), (5, 6), (6, 8), (7, 12)):
                    if tk + m < NT:
                        nc.tensor.matmul(
                            out2_all[:, 128 * (tk + m):128 * (tk + m + 1)],
                            Vn[:, tk, :], Pk[:, ri, :],
                            start=False, stop=False, skip_group_check=True)

                # bank-aligned epilogue: chunk c = cols [512c, 512c+512) is
                # complete after iteration 4c+3
                if tk % 4 == 3:
                    c = tk // 4
                    csl = slice(512 * c, 512 * (c + 1))
                    nc.vector.tensor_mul(zX[:, csl], out2_all[:, csl],
                                         gT[:, csl])

            # ---------- AllToAll exchange ----------
            bin_ = DR.tile([NC, HD + 1, NB], FP)
            bout_ = DR.tile([NC, HD + 1, NB], FP)
            for j in range(NC):
                nc.sync.dma_start(out=bin_[j],
                                  in_=zX[:, NB * j:NB * (j + 1)])
            nc.gpsimd.collective_compute(
                "AllToAll", mybir.AluOpType.bypass,
                replica_groups=[list(range(NC))],
                ins=[bin_[:].opt()], outs=[bout_[:].opt()],
            )

            # ---------- stage 3: normalize + out projection ----------
            zr = S.tile([HD, NC, NB], FP)
            for src in range(NC):
                nc.sync.dma_start(out=zr[:, src, :], in_=bout_[src, 0:HD, :])
            dens = S.tile([NC, NB], FP)
            nc.sync.dma_start(out=dens[:], in_=bout_[:, HD, :])
            recs = S.tile([NC, NB], FP)
            nc.vector.reciprocal(recs[:], dens[:])
            rdr = DR.tile([NC, NB], FP)
            nc.sync.dma_start(out=rdr[:], in_=recs[:])
            rb = S.tile([HD, NC, NB], FP)
            rdr_ap = rdr[:]
            rb_src = bass.AP(tensor=rdr_ap.tensor, offset=rdr_ap.offset,
                             ap=[[0, HD], *rdr_ap.ap])
            nc.sync.dma_start(out=rb[:], in_=rb_src)
            zn = S.tile([HD, NC, NB], FR)
            nc.vector.tensor_mul(zn[:], zr[:], rb[:])

            for nt in range(NB // 128):
                nsl = slice(128 * nt, 128 * (nt + 1))
                psY = PS.tile([128, D], FP, tag="mma")
                for h in range(H):
                    nc.tensor.matmul(psY[:], zn[:, h, nsl], wos[:, h, :],
                                     start=(h == 0), stop=False)
                nc.tensor.matmul(psY[:], ones_r[:, 0:128], bos[:],
                                 start=False, stop=True)
                ysb = W.tile([128, D], FP, tag="ysb")
                nc.scalar.copy(ysb[:], psY[:])
                nc.sync.dma_start(out=yout[nsl, :], in_=ysb[:])

    nc.compile()
    return nc


def _prep_inputs(x, W_qkv, b_qkv, W_out, b_out, W_gate, b_gate,
                 pos_bias, scale_embed, if_gain, disp_amp):
    assert not np.any(np.asarray(scale_embed)), \
        "kernel fast path requires scale_embed == 0"
    xTn = np.ascontiguousarray(np.asarray(x)[0].T.astype(np.float32))  # [D, N]
    W_qkv = np.asarray(W_qkv, dtype=np.float32)
    b_qkv = np.asarray(b_qkv, dtype=np.float32)
    W_gate = np.asarray(W_gate, dtype=np.float32)
    b_gate = np.asarray(b_gate, dtype=np.float32)
    W_out = np.asarray(W_out, dtype=np.float32)
    b_out = np.asarray(b_out, dtype=np.float32)
    pos_bias = np.asarray(pos_bias, dtype=np.float32)
    if_gain = np.asarray(if_gain, dtype=np.float32)
    disp_amp = np.asarray(disp_amp, dtype=np.float32)

    scl = 1.0 / math.sqrt(HD)
    woutS = np.ascontiguousarray(
        W_out.reshape(H, HD, D).transpose(1, 0, 2))  # [HD, H, D]

    in_maps = []
    for h in range(NC):
        qs = slice(HD * h, HD * (h + 1))
        ks = slice(D + HD * h, D + HD * (h + 1))
        vs = slice(2 * D + HD * h, 2 * D + HD * (h + 1))
        wq = W_qkv[:, qs] * scl
        wk = W_qkv[:, ks]
        wv = W_qkv[:, vs] * if_gain[h]
        wg = W_gate[:, qs]
        bq = b_qkv[qs] * scl
        bk = b_qkv[ks]
        bv = b_qkv[vs] * if_gain[h]
        bg = b_gate[qs]
        eff_pb_h = pos_bias[:, h] + DISP_COS_KERNEL[:, h] * disp_amp[h]
        in_maps.append({
            "xT": xTn,
            "wA": np.ascontiguousarray(np.concatenate([wq, wk], axis=1)),
            "wB": np.ascontiguousarray(np.concatenate([wv, wg], axis=1)),
            "bA": np.ascontiguousarray(np.concatenate([bq, bk])),
            "bB": np.ascontiguousarray(np.concatenate([bv, bg])),
            "maskW": _build_masks(eff_pb_h),
            "woutS": woutS,
            "bout": b_out,
        })
    return in_maps


def kernel(**inputs) -> np.ndarray:
    if "nc" not in _cache:
        _cache["nc"] = _build_module()
    nc = _cache["nc"]
    in_maps = _prep_inputs(**inputs)
    res = run_bass_kernel_spmd(nc, in_maps, core_ids=list(range(NC)))
    y = np.concatenate([res.results[c]["y"] for c in range(NC)], axis=0)
    return y.reshape(B, N, D)


# revision 15
# speedup vs baseline: 1.7120x; 1.7120x over previous
"""Trainium2 Bass kernel for nn_DSQGAttentionQW (sparse offset attention).

Sharding: head-tensor-parallel attention (8 heads -> 8 cores) + AllToAll
re-shard to sequence-parallel for the output projection. Single NEFF launch.
"""
import math
import numpy as np

import concourse.bacc as bacc
import concourse.bass as bass
import concourse.tile as tile
import concourse.mybir as mybir
import concourse.masks as masks
from concourse.bass_utils import run_bass_kernel_spmd

# ---- problem constants (must match reference.py) ----
_DENSE_LOCAL_W = 32
_DYADIC = [48, 64, 96, 128, 192, 256, 384, 512, 768, 1024, 1536, 2048, 3072, 4096]
OFFSETS = np.array(
    sorted(set(range(0, _DENSE_LOCAL_W + 1)) | set(_DYADIC)), dtype=np.int32
)  # [47]
NUM_OFFSETS = len(OFFSETS)
H = 8
_LOG_MAX = math.log(1.0 + 4096.0)
_HEAD_OMEGAS = [0.0, 0.0, 1 * math.pi / _LOG_MAX, 1 * math.pi / _LOG_MAX,
                4 * math.pi / _LOG_MAX, 4 * math.pi / _LOG_MAX,
                6 * math.pi / _LOG_MAX, 6 * math.pi / _LOG_MAX]
_log_d = np.log(1.0 + OFFSETS.astype(np.float64))
DISP_COS_KERNEL = np.zeros((NUM_OFFSETS, H), dtype=np.float32)
for _h, _om in enumerate(_HEAD_OMEGAS):
    if _om > 0.0:
        DISP_COS_KERNEL[:, _h] = np.cos(_om * _log_d)

B, N, D = 1, 2048, 512
HD = D // H
NC = 8
NB = N // NC            # 256: per-core output row block
NT = N // 128           # 16 q-tiles of 128
# Effective k-tile depths m (delta in (128(m-1), 128m]) that can be causal for
# N=2048: depths 16/24/32 (delta >= 2048) are never valid.
R_DEPTHS = [0, 1, 2, 3, 4, 6, 8, 12]
NR = len(R_DEPTHS)

FP = mybir.dt.float32
FR = mybir.dt.float32r
BF = mybir.dt.bfloat16

_cache = {}


def _build_masks(eff_pb_h: np.ndarray) -> np.ndarray:
    """maskW[ri, kp, i] = exp(eff_pb[offset_idx(delta)]) if delta valid else 0,
    with delta = i - kp + 128*m for depth m = R_DEPTHS[ri]."""
    off_idx = {int(d): i for i, d in enumerate(OFFSETS)}
    kp = np.arange(128)[None, :, None]
    i = np.arange(128)[None, None, :]
    m = np.array(R_DEPTHS)[:, None, None]
    delta = i - kp + 128 * m  # [NR, 128, 128]
    w = np.zeros((NR, 128, 128), dtype=np.float32)
    for d, oi in off_idx.items():
        sel = delta == d
        if sel.any():
            w[sel] = math.exp(float(eff_pb_h[oi]))
    return w


def _r(ap):
    return ap.bitcast(FR)


def _build_module():
    nc = bacc.Bacc("TRN2", target_bir_lowering=False, debug=False, num_devices=NC)

    xT = nc.dram_tensor("xT", [D, N], FR, kind="ExternalInput").ap()
    wA = nc.dram_tensor("wA", [D, 128], FR, kind="ExternalInput").ap()   # [Wq|Wk]
    wB = nc.dram_tensor("wB", [D, 128], FR, kind="ExternalInput").ap()   # [Wv|Wg]
    bA = nc.dram_tensor("bA", [128], FP, kind="ExternalInput").ap()
    bB = nc.dram_tensor("bB", [128], FP, kind="ExternalInput").ap()
    maskW = nc.dram_tensor("maskW", [NR, 128, 128], FP, kind="ExternalInput").ap()
    woutS = nc.dram_tensor("woutS", [HD, H, D], FR, kind="ExternalInput").ap()
    bout = nc.dram_tensor("bout", [D], FR, kind="ExternalInput").ap()
    yout = nc.dram_tensor("y", [NB, D], FP, kind="ExternalOutput").ap()

    with tile.TileContext(nc) as tc:
        with (
            tc.tile_pool(name="singles", bufs=1) as S,
            tc.tile_pool(name="work", bufs=3) as W,
            tc.tile_pool(name="pk", bufs=3) as PK,
            tc.tile_pool(name="ps", bufs=1, space="PSUM") as PS,
            tc.tile_pool(name="ps3", bufs=3, space="PSUM") as PS3,
            tc.tile_pool(name="pso", bufs=1, space="PSUM") as PSO,
            tc.tile_pool(name="dram", bufs=1, space="DRAM") as DR,
        ):
            # ---------- constants / loads ----------
            ident = S.tile([128, 128], FP)
            masks.make_identity(nc, ident[:])
            ones_f = S.tile([1, 128], FP)
            nc.vector.memset(ones_f[:], 1.0)
            ones_r = S.tile([1, 128], FR)
            nc.vector.tensor_copy(ones_r[:], ones_f[:])

            xs = S.tile([128, 4, N], FR)
            xT_r = xT.rearrange("(ct p) n -> p ct n", p=128)
            for ct in range(4):
                nc.sync.dma_start(out=xs[:, ct, :], in_=xT_r[:, ct, :])

            wAs = S.tile([128, 4, 128], FR)
            nc.sync.dma_start(out=wAs[:], in_=wA.rearrange("(ct p) o -> p ct o", p=128))
            wBs = S.tile([128, 4, 128], FR)
            nc.sync.dma_start(out=wBs[:], in_=wB.rearrange("(ct p) o -> p ct o", p=128))
            bAs = S.tile([128, 1], FP)
            nc.sync.dma_start(out=bAs[:], in_=bA[:, None])
            bBs = S.tile([128, 1], FP)
            nc.sync.dma_start(out=bBs[:], in_=bB[:, None])
            mws = S.tile([128, NR, 128], FP)
            nc.sync.dma_start(out=mws[:], in_=maskW.rearrange("r kp i -> kp r i"))
            wos = S.tile([HD, H, D], FR)
            nc.sync.dma_start(out=wos[:], in_=woutS[:])
            bos = S.tile([1, D], FR)
            nc.sync.dma_start(out=bos[:], in_=bout[None, :])

            # ---------- MM-A: qT / kT / vT / gT (fp32r, N=512) ----------
            qT = S.tile([64, N], FR)       # pre-scaled by 1/sqrt(HD)
            kT = S.tile([64, N], FR)
            vT = S.tile([64, N], FP)       # if_gain folded
            gT = S.tile([HD + 1, N], FP)   # sigmoid gate; row 64 = 1.0 (denom)
            nc.vector.memset(gT[HD:HD + 1, :], 1.0)
            for nch in range(4):
                nsl = slice(512 * nch, 512 * (nch + 1))
                psA = PS.tile([128, 512], FP, tag="mma")
                psB = PS.tile([128, 512], FP, tag="mma")
                for ct in range(4):
                    nc.tensor.matmul(psA[:], wAs[:, ct, :], xs[:, ct, nsl],
                                     start=(ct == 0), stop=(ct == 3))
                for ct in range(4):
                    nc.tensor.matmul(psB[:], wBs[:, ct, :], xs[:, ct, nsl],
                                     start=(ct == 0), stop=(ct == 3))
                # biased copies: q,k,v on DVE; gate sigmoid on ACT
                nc.vector.tensor_scalar_add(qT[:, nsl], psA[0:64, :], bAs[0:64])
                nc.vector.tensor_scalar_add(kT[:, nsl], psA[64:128, :], bAs[64:128])
                nc.vector.tensor_scalar_add(vT[:, nsl], psB[0:64, :], bBs[0:64])
                nc.scalar.activation(gT[0:HD, nsl], psB[64:128, :],
                                     mybir.ActivationFunctionType.Sigmoid,
                                     bias=bBs[64:128], scale=1.0)

            # ---------- V natural (fp32r) + ones column ----------
            Vn = S.tile([128, NT, HD + 1], FR)
            onesNT = S.tile([128, NT, 1], FP)
            nc.vector.memset(onesNT[:], 1.0)
            nc.vector.tensor_copy(Vn[:, :, HD:HD + 1], onesNT[:])
            for t in range(NT):
                psT = PS3.tile([128, 64], FP, tag="s")
                nc.tensor.transpose(psT[:], vT[:, 128 * t:128 * (t + 1)],
                                    ident[0:64, 0:64])
                nc.vector.tensor_copy(Vn[:, t, 0:HD], psT[:])

            # ---------- attention (k-tile-major, fp32r) ----------
            # out2_all: [65, N] PSUM accumulator (4 banks). Zero-init via K=1
            # start=True matmuls so subsequent MMs accumulate via has_written.
            out2_all = PSO.tile([HD + 1, N], FP)
            zrow_f = S.tile([1, 512], FP)
            nc.vector.memset(zrow_f[:], 0.0)
            zrow = S.tile([1, 512], FR)
            nc.vector.tensor_copy(zrow[:], zrow_f[:])
            z65 = S.tile([1, HD + 1], FR)
            nc.vector.tensor_copy(z65[:], zrow_f[:, 0:HD + 1])
            for c in range(4):
                nc.tensor.matmul(out2_all[:, 512 * c:512 * (c + 1)], z65[:],
                                 zrow[:], start=True, stop=False)

            zX = S.tile([HD + 1, N], FP)  # gated attn out^T + denom row

            Pks = [None] * NT

            def emit_mm1(tk):
                ksl = slice(128 * tk, 128 * (tk + 1))
                groups = []
                nq_near = min(3, NT - tk)
                groups.append(([0, 1, 2][:nq_near], tk, nq_near))
                nq34 = max(0, min(2, NT - tk - 3))
                if nq34:
                    groups.append(([3, 4][:nq34], tk + 3, nq34))
                if tk + 8 < NT:
                    groups.append(([6, None, 8], tk + 6, 3))
                elif tk + 6 < NT:
                    groups.append(([6], tk + 6, 1))
                if tk + 12 < NT:
                    groups.append(([12], tk + 12, 1))
                Pk = PK.tile([128, NR, 128], FR, tag="Pk")
                Pks[tk] = Pk
                for ms, q_lo, nq in groups:
                    qsl = slice(128 * q_lo, 128 * (q_lo + nq))
                    psS = PS3.tile([128, 384], FP, tag="s")
                    nc.tensor.matmul(psS[:, 0:128 * nq], kT[:, ksl],
                                     qT[:, qsl], start=True, stop=True)
                    expS = W.tile([128, 384], FP, tag="expS")
                    nc.scalar.activation(expS[:, 0:128 * nq], psS[:, 0:128 * nq],
                                         mybir.ActivationFunctionType.Exp)
                    ris = [R_DEPTHS.index(m) for m in ms if m is not None]
                    if ms == [6, None, 8]:
                        b = expS[:]
                        srcap = bass.AP(tensor=b.tensor, offset=b.offset,
                                        ap=[b.ap[0], [256, 2], [1, 128]])
                        nc.gpsimd.tensor_mul(Pk[:, ris[0]:ris[0] + 2, :],
                                             srcap, mws[:, ris[0]:ris[0] + 2, :])
                    elif ris[0] >= 5:
                        nc.gpsimd.tensor_mul(
                            Pk[:, ris[0]:ris[0] + len(ris), :],
                            expS[:, 0:128 * len(ris)],
                            mws[:, ris[0]:ris[0] + len(ris), :])
                    else:
                        nc.vector.tensor_mul(
                            Pk[:, ris[0]:ris[0] + len(ris), :],
                            expS[:, 0:128 * len(ris)],
                            mws[:, ris[0]:ris[0] + len(ris), :])

            def emit_mm2(tk):
                Pk = Pks[tk]
                nq1 = min(4, NT - tk)
                nc.tensor.matmul(
                    out2_all[:, 128 * tk:128 * (tk + nq1)],
                    Vn[:, tk, :], Pk[:, 0:nq1, :],
                    start=False, stop=(tk == NT - 1), skip_group_check=True)
                for ri, m in ((4, 4), (5, 6), (6, 8), (7, 12)):
                    if tk + m < NT:
                        nc.tensor.matmul(
                            out2_all[:, 128 * (tk + m):128 * (tk + m + 1)],
                            Vn[:, tk, :], Pk[:, ri, :],
                            start=False, stop=False, skip_group_check=True)

            def emit_epilogue(c):
                csl = slice(512 * c, 512 * (c + 1))
                nc.vector.tensor_mul(zX[:, csl], out2_all[:, csl], gT[:, csl])

            for tk in range(NT):
                emit_mm1(tk)
                if tk >= 1:
                    emit_mm2(tk - 1)
                if tk >= 5 and (tk - 5) % 4 == 0:
                    emit_epilogue((tk - 5) // 4)
            emit_mm2(NT - 1)
            emit_epilogue(2)
            emit_epilogue(3)

            # ---------- AllToAll exchange ----------
            bin_ = DR.tile([NC, HD + 1, NB], FP)
            bout_ = DR.tile([NC, HD + 1, NB], FP)
            for j in range(NC):
                nc.sync.dma_start(out=bin_[j],
                                  in_=zX[:, NB * j:NB * (j + 1)])
            nc.gpsimd.collective_compute(
                "AllToAll", mybir.AluOpType.bypass,
                replica_groups=[list(range(NC))],
                ins=[bin_[:].opt()], outs=[bout_[:].opt()],
            )

            # ---------- stage 3: normalize + out projection ----------
            zr = S.tile([HD, NC, NB], FP)
            for src in range(NC):
                nc.sync.dma_start(out=zr[:, src, :], in_=bout_[src, 0:HD, :])
            dens = S.tile([NC, NB], FP)
            nc.sync.dma_start(out=dens[:], in_=bout_[:, HD, :])
            recs = S.tile([NC, NB], FP)
            nc.vector.reciprocal(recs[:], dens[:])
            rdr = DR.tile([NC, NB], FP)
            nc.sync.dma_start(out=rdr[:], in_=recs[:])
            rb = S.tile([HD, NC, NB], FP)
            rdr_ap = rdr[:]
            rb_src = bass.AP(tensor=rdr_ap.tensor, offset=rdr_ap.offset,
                             ap=[[0, HD], *rdr_ap.ap])
            nc.sync.dma_start(out=rb[:], in_=rb_src)
            zn = S.tile([HD, NC, NB], FR)
            nc.vector.tensor_mul(zn[:], zr[:], rb[:])

            for nt in range(NB // 128):
                nsl = slice(128 * nt, 128 * (nt + 1))
                psY = PS.tile([128, D], FP, tag="mma")
                for h in range(H):
                    nc.tensor.matmul(psY[:], zn[:, h, nsl], wos[:, h, :],
                                     start=(h == 0), stop=False)
                nc.tensor.matmul(psY[:], ones_r[:, 0:128], bos[:],
                                 start=False, stop=True)
                ysb = W.tile([128, D], FP, tag="ysb")
                nc.scalar.copy(ysb[:], psY[:])
                nc.sync.dma_start(out=yout[nsl, :], in_=ysb[:])

    nc.compile()
    return nc


def _prep_inputs(x, W_qkv, b_qkv, W_out, b_out, W_gate, b_gate,
                 pos_bias, scale_embed, if_gain, disp_amp):
    assert not np.any(np.asarray(scale_embed)), \
        "kernel fast path requires scale_embed == 0"
    xTn = np.ascontiguousarray(np.asarray(x)[0].T.astype(np.float32))  # [D, N]
    W_qkv = np.asarray(W_qkv, dtype=np.float32)
    b_qkv = np.asarray(b_qkv, dtype=np.float32)
    W_gate = np.asarray(W_gate, dtype=np.float32)
    b_gate = np.asarray(b_gate, dtype=np.float32)
    W_out = np.asarray(W_out, dtype=np.float32)
    b_out = np.asarray(b_out, dtype=np.float32)
    pos_bias = np.asarray(pos_bias, dtype=np.float32)
    if_gain = np.asarray(if_gain, dtype=np.float32)
    disp_amp = np.asarray(disp_amp, dtype=np.float32)

    scl = 1.0 / math.sqrt(HD)
    woutS = np.ascontiguousarray(
        W_out.reshape(H, HD, D).transpose(1, 0, 2))  # [HD, H, D]

    in_maps = []
    for h in range(NC):
        qs = slice(HD * h, HD * (h + 1))
        ks = slice(D + HD * h, D + HD * (h + 1))
        vs = slice(2 * D + HD * h, 2 * D + HD * (h + 1))
        wq = W_qkv[:, qs] * scl
        wk = W_qkv[:, ks]
        wv = W_qkv[:, vs] * if_gain[h]
        wg = W_gate[:, qs]
        bq = b_qkv[qs] * scl
        bk = b_qkv[ks]
        bv = b_qkv[vs] * if_gain[h]
        bg = b_gate[qs]
        eff_pb_h = pos_bias[:, h] + DISP_COS_KERNEL[:, h] * disp_amp[h]
        in_maps.append({
            "xT": xTn,
            "wA": np.ascontiguousarray(np.concatenate([wq, wk], axis=1)),
            "wB": np.ascontiguousarray(np.concatenate([wv, wg], axis=1)),
            "bA": np.ascontiguousarray(np.concatenate([bq, bk])),
            "bB": np.ascontiguousarray(np.concatenate([bv, bg])),
            "maskW": _build_masks(eff_pb_h),
            "woutS": woutS,
            "bout": b_out,
        })
    return in_maps


def kernel(**inputs) -> np.ndarray:
    if "nc" not in _cache:
        _cache["nc"] = _build_module()
    nc = _cache["nc"]
    in_maps = _prep_inputs(**inputs)
    res = run_bass_kernel_spmd(nc, in_maps, core_ids=list(range(NC)))
    y = np.concatenate([res.results[c]["y"] for c in range(NC)], axis=0)
    return y.reshape(B, N, D)


# revision 17
# speedup vs baseline: 1.9017x; 1.1108x over previous
"""Trainium2 Bass kernel for nn_DSQGAttentionQW (sparse offset attention).

Sharding: head-tensor-parallel attention (8 heads -> 8 cores) + AllToAll
re-shard to sequence-parallel for the output projection. Single NEFF launch.
"""
import math
import numpy as np

import concourse.bacc as bacc
import concourse.bass as bass
import concourse.tile as tile
import concourse.mybir as mybir
import concourse.masks as masks
from concourse.bass_utils import run_bass_kernel_spmd

# ---- problem constants (must match reference.py) ----
_DENSE_LOCAL_W = 32
_DYADIC = [48, 64, 96, 128, 192, 256, 384, 512, 768, 1024, 1536, 2048, 3072, 4096]
OFFSETS = np.array(
    sorted(set(range(0, _DENSE_LOCAL_W + 1)) | set(_DYADIC)), dtype=np.int32
)  # [47]
NUM_OFFSETS = len(OFFSETS)
H = 8
_LOG_MAX = math.log(1.0 + 4096.0)
_HEAD_OMEGAS = [0.0, 0.0, 1 * math.pi / _LOG_MAX, 1 * math.pi / _LOG_MAX,
                4 * math.pi / _LOG_MAX, 4 * math.pi / _LOG_MAX,
                6 * math.pi / _LOG_MAX, 6 * math.pi / _LOG_MAX]
_log_d = np.log(1.0 + OFFSETS.astype(np.float64))
DISP_COS_KERNEL = np.zeros((NUM_OFFSETS, H), dtype=np.float32)
for _h, _om in enumerate(_HEAD_OMEGAS):
    if _om > 0.0:
        DISP_COS_KERNEL[:, _h] = np.cos(_om * _log_d)

B, N, D = 1, 2048, 512
HD = D // H
NC = 8
NB = N // NC            # 256: per-core output row block
NT = N // 128           # 16 q-tiles of 128
# Effective k-tile depths m (delta in (128(m-1), 128m]) that can be causal for
# N=2048: depths 16/24/32 (delta >= 2048) are never valid.
R_DEPTHS = [0, 1, 2, 3, 4, 6, 8, 12]
NR = len(R_DEPTHS)

FP = mybir.dt.float32
FR = mybir.dt.float32r
F16 = mybir.dt.float16

_cache = {}


def _build_masks(eff_pb_h: np.ndarray) -> np.ndarray:
    """maskW[ri, kp, i] = exp(eff_pb[offset_idx(delta)]) if delta valid else 0,
    with delta = i - kp + 128*m for depth m = R_DEPTHS[ri]."""
    off_idx = {int(d): i for i, d in enumerate(OFFSETS)}
    kp = np.arange(128)[None, :, None]
    i = np.arange(128)[None, None, :]
    m = np.array(R_DEPTHS)[:, None, None]
    delta = i - kp + 128 * m  # [NR, 128, 128]
    w = np.zeros((NR, 128, 128), dtype=np.float32)
    for d, oi in off_idx.items():
        sel = delta == d
        if sel.any():
            w[sel] = math.exp(float(eff_pb_h[oi]))
    return w


def _r(ap):
    return ap.bitcast(FR)


def _build_module():
    nc = bacc.Bacc("TRN2", target_bir_lowering=False, debug=False, num_devices=NC)

    xT = nc.dram_tensor("xT", [D, N], F16, kind="ExternalInput").ap()
    wA = nc.dram_tensor("wA", [D, 128], F16, kind="ExternalInput").ap()   # [Wq|Wk]
    wB = nc.dram_tensor("wB", [D, 128], F16, kind="ExternalInput").ap()   # [Wv|Wg]
    bA = nc.dram_tensor("bA", [128], FP, kind="ExternalInput").ap()
    bB = nc.dram_tensor("bB", [128], FP, kind="ExternalInput").ap()
    maskW = nc.dram_tensor("maskW", [NR, 128, 128], F16, kind="ExternalInput").ap()
    woutS = nc.dram_tensor("woutS", [HD, H, D], F16, kind="ExternalInput").ap()
    bout = nc.dram_tensor("bout", [D], F16, kind="ExternalInput").ap()
    yout = nc.dram_tensor("y", [NB, D], FP, kind="ExternalOutput").ap()

    with tile.TileContext(nc) as tc:
        with (
            tc.tile_pool(name="singles", bufs=1) as S,
            tc.tile_pool(name="work", bufs=3) as W,
            tc.tile_pool(name="pk", bufs=3) as PK,
            tc.tile_pool(name="ps", bufs=1, space="PSUM") as PS,
            tc.tile_pool(name="ps3", bufs=3, space="PSUM") as PS3,
            tc.tile_pool(name="pso", bufs=1, space="PSUM") as PSO,
            tc.tile_pool(name="dram", bufs=1, space="DRAM") as DR,
        ):
            # ---------- constants / loads ----------
            ident = S.tile([128, 128], F16)
            masks.make_identity(nc, ident[:])
            ones_r = S.tile([1, 128], F16)
            nc.vector.memset(ones_r[:], 1.0)

            xs = S.tile([128, 4, N], F16)
            xT_r = xT.rearrange("(ct p) n -> p ct n", p=128)
            for ct in range(4):
                nc.sync.dma_start(out=xs[:, ct, :], in_=xT_r[:, ct, :])

            wAs = S.tile([128, 4, 128], F16)
            nc.sync.dma_start(out=wAs[:], in_=wA.rearrange("(ct p) o -> p ct o", p=128))
            wBs = S.tile([128, 4, 128], F16)
            nc.sync.dma_start(out=wBs[:], in_=wB.rearrange("(ct p) o -> p ct o", p=128))
            bAs = S.tile([128, 1], FP)
            nc.sync.dma_start(out=bAs[:], in_=bA[:, None])
            bBs = S.tile([128, 1], FP)
            nc.sync.dma_start(out=bBs[:], in_=bB[:, None])
            mws = S.tile([128, NR, 128], F16)
            nc.sync.dma_start(out=mws[:], in_=maskW.rearrange("r kp i -> kp r i"))
            wos = S.tile([HD, H, D], F16)
            nc.sync.dma_start(out=wos[:], in_=woutS[:])
            bos = S.tile([1, D], F16)
            nc.sync.dma_start(out=bos[:], in_=bout[None, :])

            # ---------- MM-A: qT / kT / vT / gT (fp32r, N=512) ----------
            qT = S.tile([64, N], F16)       # pre-scaled by 1/sqrt(HD)
            kT = S.tile([64, N], F16)
            vT = S.tile([64, N], F16)       # if_gain folded
            gT = S.tile([HD + 1, N], FP)   # sigmoid gate; row 64 = 1.0 (denom)
            nc.vector.memset(gT[HD:HD + 1, :], 1.0)
            for nch in range(4):
                nsl = slice(512 * nch, 512 * (nch + 1))
                psA = PS.tile([128, 512], FP, tag="mma")
                psB = PS.tile([128, 512], FP, tag="mma")
                for ct in range(4):
                    nc.tensor.matmul(psA[:], wAs[:, ct, :], xs[:, ct, nsl],
                                     start=(ct == 0), stop=(ct == 3))
                for ct in range(4):
                    nc.tensor.matmul(psB[:], wBs[:, ct, :], xs[:, ct, nsl],
                                     start=(ct == 0), stop=(ct == 3))
                # biased copies: q,k,v on DVE; gate sigmoid on ACT
                nc.vector.tensor_scalar_add(qT[:, nsl], psA[0:64, :], bAs[0:64])
                nc.vector.tensor_scalar_add(kT[:, nsl], psA[64:128, :], bAs[64:128])
                nc.vector.tensor_scalar_add(vT[:, nsl], psB[0:64, :], bBs[0:64])
                nc.scalar.activation(gT[0:HD, nsl], psB[64:128, :],
                                     mybir.ActivationFunctionType.Sigmoid,
                                     bias=bBs[64:128], scale=1.0)

            # ---------- V natural (fp32r) + ones column ----------
            Vn = S.tile([128, NT, HD + 1], F16)
            nc.vector.memset(Vn[:, :, HD:HD + 1], 1.0)
            for t in range(NT):
                psT = PS3.tile([128, 64], F16, tag="s")
                nc.tensor.transpose(psT[:], vT[:, 128 * t:128 * (t + 1)],
                                    ident[0:64, 0:64])
                nc.vector.tensor_copy(Vn[:, t, 0:HD], psT[:])

            # ---------- attention (k-tile-major, fp32r) ----------
            # out2_all: [65, N] PSUM accumulator (4 banks). Zero-init via K=1
            # start=True matmuls so subsequent MMs accumulate via has_written.
            out2_all = PSO.tile([HD + 1, N], FP)
            zrow = S.tile([1, 512], F16)
            nc.vector.memset(zrow[:], 0.0)
            z65 = S.tile([1, HD + 1], F16)
            nc.vector.memset(z65[:], 0.0)
            for c in range(4):
                nc.tensor.matmul(out2_all[:, 512 * c:512 * (c + 1)], z65[:],
                                 zrow[:], start=True, stop=False)

            zX = S.tile([HD + 1, N], FP)  # gated attn out^T + denom row

            Pks = [None] * NT

            def emit_mm1(tk):
                ksl = slice(128 * tk, 128 * (tk + 1))
                groups = []
                nq_near = min(3, NT - tk)
                groups.append(([0, 1, 2][:nq_near], tk, nq_near))
                nq34 = max(0, min(2, NT - tk - 3))
                if nq34:
                    groups.append(([3, 4][:nq34], tk + 3, nq34))
                if tk + 8 < NT:
                    groups.append(([6, None, 8], tk + 6, 3))
                elif tk + 6 < NT:
                    groups.append(([6], tk + 6, 1))
                if tk + 12 < NT:
                    groups.append(([12], tk + 12, 1))
                Pk = PK.tile([128, NR, 128], F16, tag="Pk")
                Pks[tk] = Pk
                for ms, q_lo, nq in groups:
                    qsl = slice(128 * q_lo, 128 * (q_lo + nq))
                    psS = PS3.tile([128, 384], FP, tag="s")
                    nc.tensor.matmul(psS[:, 0:128 * nq], kT[:, ksl],
                                     qT[:, qsl], start=True, stop=True)
                    expS = W.tile([128, 384], FP, tag="expS")
                    nc.scalar.activation(expS[:, 0:128 * nq], psS[:, 0:128 * nq],
                                         mybir.ActivationFunctionType.Exp)
                    ris = [R_DEPTHS.index(m) for m in ms if m is not None]
                    if ms == [6, None, 8]:
                        b = expS[:]
                        srcap = bass.AP(tensor=b.tensor, offset=b.offset,
                                        ap=[b.ap[0], [256, 2], [1, 128]])
                        nc.gpsimd.tensor_mul(Pk[:, ris[0]:ris[0] + 2, :],
                                             srcap, mws[:, ris[0]:ris[0] + 2, :])
                    elif ris[0] >= 5:
                        nc.gpsimd.tensor_mul(
                            Pk[:, ris[0]:ris[0] + len(ris), :],
                            expS[:, 0:128 * len(ris)],
                            mws[:, ris[0]:ris[0] + len(ris), :])
                    else:
                        nc.vector.tensor_mul(
                            Pk[:, ris[0]:ris[0] + len(ris), :],
                            expS[:, 0:128 * len(ris)],
                            mws[:, ris[0]:ris[0] + len(ris), :])

            def emit_mm2(tk):
                Pk = Pks[tk]
                nq1 = min(4, NT - tk)
                nc.tensor.matmul(
                    out2_all[:, 128 * tk:128 * (tk + nq1)],
                    Vn[:, tk, :], Pk[:, 0:nq1, :],
                    start=False, stop=(tk == NT - 1), skip_group_check=True)
                for ri, m in ((4, 4), (5, 6), (6, 8), (7, 12)):
                    if tk + m < NT:
                        nc.tensor.matmul(
                            out2_all[:, 128 * (tk + m):128 * (tk + m + 1)],
                            Vn[:, tk, :], Pk[:, ri, :],
                            start=False, stop=False, skip_group_check=True)

            def emit_epilogue(c):
                csl = slice(512 * c, 512 * (c + 1))
                nc.vector.tensor_mul(zX[:, csl], out2_all[:, csl], gT[:, csl])

            for tk in range(NT):
                emit_mm1(tk)
                if tk >= 1:
                    emit_mm2(tk - 1)
                if tk >= 5 and (tk - 5) % 4 == 0:
                    emit_epilogue((tk - 5) // 4)
            emit_mm2(NT - 1)
            emit_epilogue(2)
            emit_epilogue(3)

            # ---------- AllToAll exchange ----------
            bin_ = DR.tile([NC, HD + 1, NB], FP)
            bout_ = DR.tile([NC, HD + 1, NB], FP)
            for j in range(NC):
                nc.sync.dma_start(out=bin_[j],
                                  in_=zX[:, NB * j:NB * (j + 1)])
            nc.gpsimd.collective_compute(
                "AllToAll", mybir.AluOpType.bypass,
                replica_groups=[list(range(NC))],
                ins=[bin_[:].opt()], outs=[bout_[:].opt()],
            )

            # ---------- stage 3: normalize + out projection ----------
            zr = S.tile([HD, NC, NB], FP)
            for src in range(NC):
                nc.sync.dma_start(out=zr[:, src, :], in_=bout_[src, 0:HD, :])
            dens = S.tile([NC, NB], FP)
            nc.sync.dma_start(out=dens[:], in_=bout_[:, HD, :])
            recs = S.tile([NC, NB], FP)
            nc.vector.reciprocal(recs[:], dens[:])
            rdr = DR.tile([NC, NB], FP)
            nc.sync.dma_start(out=rdr[:], in_=recs[:])
            rb = S.tile([HD, NC, NB], FP)
            rdr_ap = rdr[:]
            rb_src = bass.AP(tensor=rdr_ap.tensor, offset=rdr_ap.offset,
                             ap=[[0, HD], *rdr_ap.ap])
            nc.sync.dma_start(out=rb[:], in_=rb_src)
            zn = S.tile([HD, NC, NB], F16)
            nc.vector.tensor_mul(zn[:], zr[:], rb[:])

            for nt in range(NB // 128):
                nsl = slice(128 * nt, 128 * (nt + 1))
                psY = PS.tile([128, D], FP, tag="mma")
                for h in range(H):
                    nc.tensor.matmul(psY[:], zn[:, h, nsl], wos[:, h, :],
                                     start=(h == 0), stop=False)
                nc.tensor.matmul(psY[:], ones_r[:, 0:128], bos[:],
                                 start=False, stop=True)
                ysb = W.tile([128, D], FP, tag="ysb")
                nc.scalar.copy(ysb[:], psY[:])
                nc.sync.dma_start(out=yout[nsl, :], in_=ysb[:])

    nc.compile()
    return nc


def _prep_inputs(x, W_qkv, b_qkv, W_out, b_out, W_gate, b_gate,
                 pos_bias, scale_embed, if_gain, disp_amp):
    assert not np.any(np.asarray(scale_embed)), \
        "kernel fast path requires scale_embed == 0"
    xTn = np.ascontiguousarray(np.asarray(x)[0].T.astype(np.float32))  # [D, N]
    W_qkv = np.asarray(W_qkv, dtype=np.float32)
    b_qkv = np.asarray(b_qkv, dtype=np.float32)
    W_gate = np.asarray(W_gate, dtype=np.float32)
    b_gate = np.asarray(b_gate, dtype=np.float32)
    W_out = np.asarray(W_out, dtype=np.float32)
    b_out = np.asarray(b_out, dtype=np.float32)
    pos_bias = np.asarray(pos_bias, dtype=np.float32)
    if_gain = np.asarray(if_gain, dtype=np.float32)
    disp_amp = np.asarray(disp_amp, dtype=np.float32)

    scl = 1.0 / math.sqrt(HD)
    wout16 = np.ascontiguousarray(
        W_out.reshape(H, HD, D).transpose(1, 0, 2)).astype(np.float16)  # [HD,H,D]
    xT16 = xTn.astype(np.float16)

    in_maps = []
    for h in range(NC):
        qs = slice(HD * h, HD * (h + 1))
        ks = slice(D + HD * h, D + HD * (h + 1))
        vs = slice(2 * D + HD * h, 2 * D + HD * (h + 1))
        wq = W_qkv[:, qs] * scl
        wk = W_qkv[:, ks]
        wv = W_qkv[:, vs] * if_gain[h]
        wg = W_gate[:, qs]
        bq = b_qkv[qs] * scl
        bk = b_qkv[ks]
        bv = b_qkv[vs] * if_gain[h]
        bg = b_gate[qs]
        eff_pb_h = pos_bias[:, h] + DISP_COS_KERNEL[:, h] * disp_amp[h]
        in_maps.append({
            "xT": xT16,
            "wA": np.ascontiguousarray(
                np.concatenate([wq, wk], axis=1)).astype(np.float16),
            "wB": np.ascontiguousarray(
                np.concatenate([wv, wg], axis=1)).astype(np.float16),
            "bA": np.ascontiguousarray(np.concatenate([bq, bk])),
            "bB": np.ascontiguousarray(np.concatenate([bv, bg])),
            "maskW": _build_masks(eff_pb_h).astype(np.float16),
            "woutS": wout16,
            "bout": b_out.astype(np.float16),
        })
    return in_maps


def kernel(**inputs) -> np.ndarray:
    if "nc" not in _cache:
        _cache["nc"] = _build_module()
    nc = _cache["nc"]
    in_maps = _prep_inputs(**inputs)
    res = run_bass_kernel_spmd(nc, in_maps, core_ids=list(range(NC)))
    y = np.concatenate([res.results[c]["y"] for c in range(NC)], axis=0)
    return y.reshape(B, N, D)


# revision 19
# speedup vs baseline: 2.0175x; 1.0609x over previous
"""Trainium2 Bass kernel for nn_DSQGAttentionQW (sparse offset attention).

Sharding: head-tensor-parallel attention (8 heads -> 8 cores) + AllToAll
re-shard to sequence-parallel for the output projection. Single NEFF launch.
"""
import math
import numpy as np

import concourse.bacc as bacc
import concourse.bass as bass
import concourse.tile as tile
import concourse.mybir as mybir
import concourse.masks as masks
from concourse.bass_utils import run_bass_kernel_spmd

# ---- problem constants (must match reference.py) ----
_DENSE_LOCAL_W = 32
_DYADIC = [48, 64, 96, 128, 192, 256, 384, 512, 768, 1024, 1536, 2048, 3072, 4096]
OFFSETS = np.array(
    sorted(set(range(0, _DENSE_LOCAL_W + 1)) | set(_DYADIC)), dtype=np.int32
)  # [47]
NUM_OFFSETS = len(OFFSETS)
H = 8
_LOG_MAX = math.log(1.0 + 4096.0)
_HEAD_OMEGAS = [0.0, 0.0, 1 * math.pi / _LOG_MAX, 1 * math.pi / _LOG_MAX,
                4 * math.pi / _LOG_MAX, 4 * math.pi / _LOG_MAX,
                6 * math.pi / _LOG_MAX, 6 * math.pi / _LOG_MAX]
_log_d = np.log(1.0 + OFFSETS.astype(np.float64))
DISP_COS_KERNEL = np.zeros((NUM_OFFSETS, H), dtype=np.float32)
for _h, _om in enumerate(_HEAD_OMEGAS):
    if _om > 0.0:
        DISP_COS_KERNEL[:, _h] = np.cos(_om * _log_d)

B, N, D = 1, 2048, 512
HD = D // H
NC = 8
NB = N // NC            # 256: per-core output row block
NT = N // 128           # 16 q-tiles of 128
# Effective k-tile depths m (delta in (128(m-1), 128m]) that can be causal for
# N=2048: depths 16/24/32 (delta >= 2048) are never valid.
R_DEPTHS = [0, 1, 2, 3, 4, 6, 8, 12]
NR = len(R_DEPTHS)

FP = mybir.dt.float32
FR = mybir.dt.float32r
F16 = mybir.dt.float16

_cache = {}


def _build_masks(eff_pb_h: np.ndarray) -> np.ndarray:
    """maskW[ri, kp, i] = exp(eff_pb[offset_idx(delta)]) if delta valid else 0,
    with delta = i - kp + 128*m for depth m = R_DEPTHS[ri]."""
    off_idx = {int(d): i for i, d in enumerate(OFFSETS)}
    kp = np.arange(128)[None, :, None]
    i = np.arange(128)[None, None, :]
    m = np.array(R_DEPTHS)[:, None, None]
    delta = i - kp + 128 * m  # [NR, 128, 128]
    w = np.zeros((NR, 128, 128), dtype=np.float32)
    for d, oi in off_idx.items():
        sel = delta == d
        if sel.any():
            w[sel] = math.exp(float(eff_pb_h[oi]))
    return w


def _r(ap):
    return ap.bitcast(FR)


def _build_module():
    nc = bacc.Bacc("TRN2", target_bir_lowering=False, debug=False, num_devices=NC)

    xT = nc.dram_tensor("xT", [D, N], F16, kind="ExternalInput").ap()
    wA = nc.dram_tensor("wA", [D, 128], F16, kind="ExternalInput").ap()   # [Wq|Wk]
    wB = nc.dram_tensor("wB", [D, 128], F16, kind="ExternalInput").ap()   # [Wv|Wg]
    bA = nc.dram_tensor("bA", [128], FP, kind="ExternalInput").ap()
    bB = nc.dram_tensor("bB", [128], FP, kind="ExternalInput").ap()
    maskW = nc.dram_tensor("maskW", [NR, 128, 128], F16, kind="ExternalInput").ap()
    woutS = nc.dram_tensor("woutS", [128, 4, D], F16, kind="ExternalInput").ap()
    bout = nc.dram_tensor("bout", [D], F16, kind="ExternalInput").ap()
    yout = nc.dram_tensor("y", [NB, D], FP, kind="ExternalOutput").ap()

    with tile.TileContext(nc) as tc:
        with (
            tc.tile_pool(name="singles", bufs=1) as S,
            tc.tile_pool(name="work", bufs=3) as W,
            tc.tile_pool(name="pk", bufs=3) as PK,
            tc.tile_pool(name="ps", bufs=1, space="PSUM") as PS,
            tc.tile_pool(name="ps3", bufs=3, space="PSUM") as PS3,
            tc.tile_pool(name="pso", bufs=1, space="PSUM") as PSO,
            tc.tile_pool(name="dram", bufs=1, space="DRAM") as DR,
        ):
            # ---------- constants / loads ----------
            ident = S.tile([128, 128], F16)
            masks.make_identity(nc, ident[:])
            ones_r = S.tile([1, 128], F16)
            nc.vector.memset(ones_r[:], 1.0)

            xs = S.tile([128, 4, N], F16)
            xT_r = xT.rearrange("(ct p) n -> p ct n", p=128)
            for ct in range(4):
                nc.sync.dma_start(out=xs[:, ct, :], in_=xT_r[:, ct, :])

            wAs = S.tile([128, 4, 128], F16)
            nc.sync.dma_start(out=wAs[:], in_=wA.rearrange("(ct p) o -> p ct o", p=128))
            wBs = S.tile([128, 4, 128], F16)
            nc.sync.dma_start(out=wBs[:], in_=wB.rearrange("(ct p) o -> p ct o", p=128))
            bAs = S.tile([128, 1], FP)
            nc.sync.dma_start(out=bAs[:], in_=bA[:, None])
            bBs = S.tile([128, 1], FP)
            nc.sync.dma_start(out=bBs[:], in_=bB[:, None])
            mws = S.tile([128, NR, 128], F16)
            nc.sync.dma_start(out=mws[:], in_=maskW.rearrange("r kp i -> kp r i"))
            wos = S.tile([128, 4, D], F16)
            nc.sync.dma_start(out=wos[:], in_=woutS[:])
            bos = S.tile([1, D], F16)
            nc.sync.dma_start(out=bos[:], in_=bout[None, :])

            # ---------- MM-A: qT / kT / vT / gT (fp32r, N=512) ----------
            qT = S.tile([64, N], F16)       # pre-scaled by 1/sqrt(HD)
            kT = S.tile([64, N], F16)
            vT = S.tile([64, N], F16)       # if_gain folded
            gT = S.tile([HD + 1, N], FP)   # sigmoid gate; row 64 = 1.0 (denom)
            nc.vector.memset(gT[HD:HD + 1, :], 1.0)
            Vn = S.tile([128, NT, HD + 1], F16)
            nc.vector.memset(Vn[:, :, HD:HD + 1], 1.0)

            def emit_transpose(t):
                psT = PS3.tile([128, 64], F16, tag="s")
                nc.tensor.transpose(psT[:], vT[:, 128 * t:128 * (t + 1)],
                                    ident[0:64, 0:64])
                nc.vector.tensor_copy(Vn[:, t, 0:HD], psT[:])

            for nch in range(4):
                nsl = slice(512 * nch, 512 * (nch + 1))
                psA = PS.tile([128, 512], FP, tag="mma")
                psB = PS3.tile([128, 512], FP, tag="s")
                for ct in range(4):
                    nc.tensor.matmul(psA[:], wAs[:, ct, :], xs[:, ct, nsl],
                                     start=(ct == 0), stop=(ct == 3))
                for ct in range(4):
                    nc.tensor.matmul(psB[:], wBs[:, ct, :], xs[:, ct, nsl],
                                     start=(ct == 0), stop=(ct == 3))
                # biased copies: q,k,v on DVE; gate sigmoid on ACT
                nc.vector.tensor_scalar_add(qT[:, nsl], psA[0:64, :], bAs[0:64])
                nc.vector.tensor_scalar_add(kT[:, nsl], psA[64:128, :], bAs[64:128])
                nc.scalar.activation(vT[:, nsl], psB[0:64, :],
                                     mybir.ActivationFunctionType.Identity,
                                     bias=bBs[0:64], scale=1.0)
                nc.scalar.activation(gT[0:HD, nsl], psB[64:128, :],
                                     mybir.ActivationFunctionType.Sigmoid,
                                     bias=bBs[64:128], scale=1.0)
                if nch >= 1:
                    for t in range(4 * (nch - 1), 4 * nch):
                        emit_transpose(t)


            for t in range(12, 16):
                emit_transpose(t)

            # ---------- attention (k-tile-major, fp16) ----------
            # out2_all: [65, N] PSUM accumulator (4 banks). Zero-init via K=1
            # start=True matmuls so subsequent MMs accumulate via has_written.
            out2_all = PSO.tile([HD + 1, N], FP)
            zrow = S.tile([1, 512], F16)
            nc.vector.memset(zrow[:], 0.0)
            z65 = S.tile([1, HD + 1], F16)
            nc.vector.memset(z65[:], 0.0)
            for c in range(4):
                nc.tensor.matmul(out2_all[:, 512 * c:512 * (c + 1)], z65[:],
                                 zrow[:], start=True, stop=False)

            zX = S.tile([HD + 1, N], FP)  # gated attn out^T + denom row

            Pks = [None] * NT

            def emit_mm1(tk):
                ksl = slice(128 * tk, 128 * (tk + 1))
                groups = []
                nq_near = min(3, NT - tk)
                groups.append(([0, 1, 2][:nq_near], tk, nq_near))
                nq34 = max(0, min(2, NT - tk - 3))
                if nq34:
                    groups.append(([3, 4][:nq34], tk + 3, nq34))
                if tk + 8 < NT:
                    groups.append(([6, None, 8], tk + 6, 3))
                elif tk + 6 < NT:
                    groups.append(([6], tk + 6, 1))
                if tk + 12 < NT:
                    groups.append(([12], tk + 12, 1))
                Pk = PK.tile([128, NR, 128], F16, tag="Pk")
                Pks[tk] = Pk
                for ms, q_lo, nq in groups:
                    qsl = slice(128 * q_lo, 128 * (q_lo + nq))
                    psS = PS3.tile([128, 384], FP, tag="s")
                    nc.tensor.matmul(psS[:, 0:128 * nq], kT[:, ksl],
                                     qT[:, qsl], start=True, stop=True)
                    expS = W.tile([128, 384], FP, tag="expS")
                    nc.scalar.activation(expS[:, 0:128 * nq], psS[:, 0:128 * nq],
                                         mybir.ActivationFunctionType.Exp)
                    ris = [R_DEPTHS.index(m) for m in ms if m is not None]
                    if ms == [6, None, 8]:
                        b = expS[:]
                        srcap = bass.AP(tensor=b.tensor, offset=b.offset,
                                        ap=[b.ap[0], [256, 2], [1, 128]])
                        nc.gpsimd.tensor_mul(Pk[:, ris[0]:ris[0] + 2, :],
                                             srcap, mws[:, ris[0]:ris[0] + 2, :])
                    elif ris[0] >= 5:
                        nc.gpsimd.tensor_mul(
                            Pk[:, ris[0]:ris[0] + len(ris), :],
                            expS[:, 0:128 * len(ris)],
                            mws[:, ris[0]:ris[0] + len(ris), :])
                    else:
                        nc.vector.tensor_mul(
                            Pk[:, ris[0]:ris[0] + len(ris), :],
                            expS[:, 0:128 * len(ris)],
                            mws[:, ris[0]:ris[0] + len(ris), :])

            def emit_mm2(tk):
                Pk = Pks[tk]
                nq1 = min(4, NT - tk)
                nc.tensor.matmul(
                    out2_all[:, 128 * tk:128 * (tk + nq1)],
                    Vn[:, tk, :], Pk[:, 0:nq1, :],
                    start=False, stop=(tk == NT - 1), skip_group_check=True)
                for ri, m in ((4, 4), (5, 6), (6, 8), (7, 12)):
                    if tk + m < NT:
                        nc.tensor.matmul(
                            out2_all[:, 128 * (tk + m):128 * (tk + m + 1)],
                            Vn[:, tk, :], Pk[:, ri, :],
                            start=False, stop=False, skip_group_check=True)

            def emit_epilogue(c):
                csl = slice(512 * c, 512 * (c + 1))
                nc.vector.tensor_mul(zX[:, csl], out2_all[:, csl], gT[:, csl])

            for tk in range(NT):
                emit_mm1(tk)
                if tk >= 1:
                    emit_mm2(tk - 1)
                if tk >= 5 and (tk - 5) % 4 == 0:
                    emit_epilogue((tk - 5) // 4)
            emit_mm2(NT - 1)
            emit_epilogue(2)
            emit_epilogue(3)

            # ---------- AllToAll exchange ----------
            bin_ = DR.tile([NC, HD + 1, NB], FP)
            bout_ = DR.tile([NC, HD + 1, NB], FP)
            for j in range(NC):
                nc.sync.dma_start(out=bin_[j],
                                  in_=zX[:, NB * j:NB * (j + 1)])
            nc.gpsimd.collective_compute(
                "AllToAll", mybir.AluOpType.bypass,
                replica_groups=[list(range(NC))],
                ins=[bin_[:].opt()], outs=[bout_[:].opt()],
            )

            # ---------- stage 3: normalize + out projection ----------
            # pair-stack heads on partitions: zr2[c, p, n] = z^T[(2p)*64+c] rows
            SRC = NC * (HD + 1) * NB   # element stride between sources in bout_
            bo = bout_[:]
            zr2 = S.tile([128, 4, NB], FP)
            for par in range(2):
                inap = bass.AP(tensor=bo.tensor,
                               offset=bo.offset + par * (HD + 1) * NB,
                               ap=[[NB, 64], [2 * (HD + 1) * NB, 4], [1, NB]])
                nc.sync.dma_start(out=zr2[64 * par:64 * (par + 1), :, :], in_=inap)
            rb2 = S.tile([128, 4, NB], FP)
            for par in range(2):
                inap = bass.AP(tensor=bo.tensor,
                               offset=bo.offset + HD * NB + par * (HD + 1) * NB,
                               ap=[[0, 64], [2 * (HD + 1) * NB, 4], [1, NB]])
                nc.sync.dma_start(out=rb2[64 * par:64 * (par + 1), :, :], in_=inap)
            rr2 = S.tile([128, 4, NB], FP)
            nc.vector.reciprocal(rr2[:], rb2[:])
            zn2 = S.tile([128, 4, NB], F16)
            nc.vector.tensor_mul(zn2[:], zr2[:], rr2[:])

            for nt in range(NB // 128):
                nsl = slice(128 * nt, 128 * (nt + 1))
                psY = PS3.tile([128, D], FP, tag="s")
                for p in range(4):
                    nc.tensor.matmul(psY[:], zn2[:, p, nsl], wos[:, p, :],
                                     start=(p == 0), stop=False)
                nc.tensor.matmul(psY[:], ones_r[:, 0:128], bos[:],
                                 start=False, stop=True)
                ysb = W.tile([128, D], FP, tag="ysb")
                nc.scalar.copy(ysb[:], psY[:])
                nc.sync.dma_start(out=yout[nsl, :], in_=ysb[:])

    nc.compile()
    return nc


def _prep_inputs(x, W_qkv, b_qkv, W_out, b_out, W_gate, b_gate,
                 pos_bias, scale_embed, if_gain, disp_amp):
    assert not np.any(np.asarray(scale_embed)), \
        "kernel fast path requires scale_embed == 0"
    xTn = np.ascontiguousarray(np.asarray(x)[0].T.astype(np.float32))  # [D, N]
    W_qkv = np.asarray(W_qkv, dtype=np.float32)
    b_qkv = np.asarray(b_qkv, dtype=np.float32)
    W_gate = np.asarray(W_gate, dtype=np.float32)
    b_gate = np.asarray(b_gate, dtype=np.float32)
    W_out = np.asarray(W_out, dtype=np.float32)
    b_out = np.asarray(b_out, dtype=np.float32)
    pos_bias = np.asarray(pos_bias, dtype=np.float32)
    if_gain = np.asarray(if_gain, dtype=np.float32)
    disp_amp = np.asarray(disp_amp, dtype=np.float32)

    scl = 1.0 / math.sqrt(HD)
    wout16 = np.ascontiguousarray(
        W_out.reshape(4, 128, D).transpose(1, 0, 2)).astype(np.float16)  # [128,4,D]
    xT16 = xTn.astype(np.float16)

    in_maps = []
    for h in range(NC):
        qs = slice(HD * h, HD * (h + 1))
        ks = slice(D + HD * h, D + HD * (h + 1))
        vs = slice(2 * D + HD * h, 2 * D + HD * (h + 1))
        wq = W_qkv[:, qs] * scl
        wk = W_qkv[:, ks]
        wv = W_qkv[:, vs] * if_gain[h]
        wg = W_gate[:, qs]
        bq = b_qkv[qs] * scl
        bk = b_qkv[ks]
        bv = b_qkv[vs] * if_gain[h]
        bg = b_gate[qs]
        eff_pb_h = pos_bias[:, h] + DISP_COS_KERNEL[:, h] * disp_amp[h]
        in_maps.append({
            "xT": xT16,
            "wA": np.ascontiguousarray(
                np.concatenate([wq, wk], axis=1)).astype(np.float16),
            "wB": np.ascontiguousarray(
                np.concatenate([wv, wg], axis=1)).astype(np.float16),
            "bA": np.ascontiguousarray(np.concatenate([bq, bk])),
            "bB": np.ascontiguousarray(np.concatenate([bv, bg])),
            "maskW": _build_masks(eff_pb_h).astype(np.float16),
            "woutS": wout16,
            "bout": b_out.astype(np.float16),
        })
    return in_maps


def kernel(**inputs) -> np.ndarray:
    if "nc" not in _cache:
        _cache["nc"] = _build_module()
    nc = _cache["nc"]
    in_maps = _prep_inputs(**inputs)
    res = run_bass_kernel_spmd(nc, in_maps, core_ids=list(range(NC)))
    y = np.concatenate([res.results[c]["y"] for c in range(NC)], axis=0)
    return y.reshape(B, N, D)


# revision 21
# speedup vs baseline: 2.1534x; 1.0673x over previous
"""Trainium2 Bass kernel for nn_DSQGAttentionQW (sparse offset attention).

Sharding: head-tensor-parallel attention (8 heads -> 8 cores) + AllToAll
re-shard to sequence-parallel for the output projection. Single NEFF launch.
"""
import math
import numpy as np

import concourse.bacc as bacc
import concourse.bass as bass
import concourse.tile as tile
import concourse.mybir as mybir
import concourse.masks as masks
from concourse.bass_utils import run_bass_kernel_spmd

# ---- problem constants (must match reference.py) ----
_DENSE_LOCAL_W = 32
_DYADIC = [48, 64, 96, 128, 192, 256, 384, 512, 768, 1024, 1536, 2048, 3072, 4096]
OFFSETS = np.array(
    sorted(set(range(0, _DENSE_LOCAL_W + 1)) | set(_DYADIC)), dtype=np.int32
)  # [47]
NUM_OFFSETS = len(OFFSETS)
H = 8
_LOG_MAX = math.log(1.0 + 4096.0)
_HEAD_OMEGAS = [0.0, 0.0, 1 * math.pi / _LOG_MAX, 1 * math.pi / _LOG_MAX,
                4 * math.pi / _LOG_MAX, 4 * math.pi / _LOG_MAX,
                6 * math.pi / _LOG_MAX, 6 * math.pi / _LOG_MAX]
_log_d = np.log(1.0 + OFFSETS.astype(np.float64))
DISP_COS_KERNEL = np.zeros((NUM_OFFSETS, H), dtype=np.float32)
for _h, _om in enumerate(_HEAD_OMEGAS):
    if _om > 0.0:
        DISP_COS_KERNEL[:, _h] = np.cos(_om * _log_d)

B, N, D = 1, 2048, 512
HD = D // H
NC = 8
NB = N // NC            # 256: per-core output row block
NT = N // 128           # 16 q-tiles of 128
# Effective k-tile depths m (delta in (128(m-1), 128m]) that can be causal for
# N=2048: depths 16/24/32 (delta >= 2048) are never valid.
R_DEPTHS = [0, 1, 2, 3, 4, 6, 8, 12]
NR = len(R_DEPTHS)

FP = mybir.dt.float32
FR = mybir.dt.float32r
F16 = mybir.dt.float16

_cache = {}


def _build_masks(eff_pb_h: np.ndarray) -> np.ndarray:
    """maskW[ri, kp, i] = exp(eff_pb[offset_idx(delta)]) if delta valid else 0,
    with delta = i - kp + 128*m for depth m = R_DEPTHS[ri]."""
    off_idx = {int(d): i for i, d in enumerate(OFFSETS)}
    kp = np.arange(128)[None, :, None]
    i = np.arange(128)[None, None, :]
    m = np.array(R_DEPTHS)[:, None, None]
    delta = i - kp + 128 * m  # [NR, 128, 128]
    w = np.zeros((NR, 128, 128), dtype=np.float32)
    for d, oi in off_idx.items():
        sel = delta == d
        if sel.any():
            w[sel] = math.exp(float(eff_pb_h[oi]))
    return w


def _r(ap):
    return ap.bitcast(FR)


def _build_module():
    nc = bacc.Bacc("TRN2", target_bir_lowering=False, debug=False, num_devices=NC)

    xT = nc.dram_tensor("xT", [D, N], F16, kind="ExternalInput").ap()
    wA = nc.dram_tensor("wA", [D, 128], F16, kind="ExternalInput").ap()   # [Wq|Wk]
    wB = nc.dram_tensor("wB", [D, 128], F16, kind="ExternalInput").ap()   # [Wv|Wg]
    bA = nc.dram_tensor("bA", [128], FP, kind="ExternalInput").ap()
    bB = nc.dram_tensor("bB", [128], FP, kind="ExternalInput").ap()
    maskW = nc.dram_tensor("maskW", [NR, 128, 128], F16, kind="ExternalInput").ap()
    woutS = nc.dram_tensor("woutS", [128, 4, D], F16, kind="ExternalInput").ap()
    bout = nc.dram_tensor("bout", [D], F16, kind="ExternalInput").ap()
    yout = nc.dram_tensor("y", [NB, D], FP, kind="ExternalOutput").ap()

    with tile.TileContext(nc) as tc:
        with (
            tc.tile_pool(name="singles", bufs=1) as S,
            tc.tile_pool(name="work", bufs=3) as W,
            tc.tile_pool(name="pk", bufs=3) as PK,
            tc.tile_pool(name="ps", bufs=1, space="PSUM") as PS,
            tc.tile_pool(name="ps3", bufs=3, space="PSUM") as PS3,
            tc.tile_pool(name="pso", bufs=1, space="PSUM") as PSO,
            tc.tile_pool(name="dram", bufs=1, space="DRAM") as DR,
        ):
            # ---------- constants / loads ----------
            ident = S.tile([128, 128], F16)
            masks.make_identity(nc, ident[:])
            ones_r = S.tile([1, 128], F16)
            nc.vector.memset(ones_r[:], 1.0)

            xs = S.tile([128, 4, N], F16)
            xT_r = xT.rearrange("(ct p) n -> p ct n", p=128)
            for nch in range(4):
                for ct in range(4):
                    nsl = slice(512 * nch, 512 * (nch + 1))
                    nc.sync.dma_start(out=xs[:, ct, nsl], in_=xT_r[:, ct, nsl])

            wAs = S.tile([128, 4, 128], F16)
            nc.sync.dma_start(out=wAs[:], in_=wA.rearrange("(ct p) o -> p ct o", p=128))
            wBs = S.tile([128, 4, 128], F16)
            nc.sync.dma_start(out=wBs[:], in_=wB.rearrange("(ct p) o -> p ct o", p=128))
            bAs = S.tile([128, 1], FP)
            nc.sync.dma_start(out=bAs[:], in_=bA[:, None])
            bBs = S.tile([128, 1], FP)
            nc.sync.dma_start(out=bBs[:], in_=bB[:, None])
            mws = S.tile([128, NR, 128], F16)
            nc.sync.dma_start(out=mws[:], in_=maskW.rearrange("r kp i -> kp r i"))
            wos = S.tile([128, 4, D], F16)
            nc.sync.dma_start(out=wos[:], in_=woutS[:])
            bos = S.tile([1, D], F16)
            nc.sync.dma_start(out=bos[:], in_=bout[None, :])

            # ---------- MM-A: qT / kT / vT / gT (fp32r, N=512) ----------
            qT = S.tile([64, N], F16)       # pre-scaled by 1/sqrt(HD)
            kT = S.tile([64, N], F16)
            vT = S.tile([64, N], F16)       # if_gain folded
            gT = S.tile([HD + 1, N], FP)   # sigmoid gate; row 64 = 1.0 (denom)
            nc.vector.memset(gT[HD:HD + 1, :], 1.0)
            Vn = S.tile([128, NT, HD + 1], F16)
            nc.vector.memset(Vn[:, :, HD:HD + 1], 1.0)

            def emit_transpose(t):
                psT = PS3.tile([128, 64], F16, tag="s")
                nc.tensor.transpose(psT[:], vT[:, 128 * t:128 * (t + 1)],
                                    ident[0:64, 0:64])
                nc.vector.tensor_copy(Vn[:, t, 0:HD], psT[:])

            for nch in range(4):
                nsl = slice(512 * nch, 512 * (nch + 1))
                psA = PS.tile([128, 512], FP, tag="mma")
                psB = PS3.tile([128, 512], FP, tag="s")
                for ct in range(4):
                    nc.tensor.matmul(psA[:], wAs[:, ct, :], xs[:, ct, nsl],
                                     start=(ct == 0), stop=(ct == 3))
                for ct in range(4):
                    nc.tensor.matmul(psB[:], wBs[:, ct, :], xs[:, ct, nsl],
                                     start=(ct == 0), stop=(ct == 3))
                # biased copies: q,k,v on DVE; gate sigmoid on ACT
                nc.vector.tensor_scalar_add(qT[:, nsl], psA[0:64, :], bAs[0:64])
                nc.vector.tensor_scalar_add(kT[:, nsl], psA[64:128, :], bAs[64:128])
                nc.scalar.activation(vT[:, nsl], psB[0:64, :],
                                     mybir.ActivationFunctionType.Identity,
                                     bias=bBs[0:64], scale=1.0)
                nc.scalar.activation(gT[0:HD, nsl], psB[64:128, :],
                                     mybir.ActivationFunctionType.Sigmoid,
                                     bias=bBs[64:128], scale=1.0)
                if nch >= 1:
                    for t in range(4 * (nch - 1), 4 * nch):
                        emit_transpose(t)


            for t in range(12, 16):
                emit_transpose(t)

            # ---------- attention (k-tile-major, fp16) ----------
            # out2_all: [65, N] PSUM accumulator (4 banks). Zero-init via K=1
            # start=True matmuls so subsequent MMs accumulate via has_written.
            out2_all = PSO.tile([HD + 1, N], FP)
            zrow = S.tile([1, 512], F16)
            nc.vector.memset(zrow[:], 0.0)
            z65 = S.tile([1, HD + 1], F16)
            nc.vector.memset(z65[:], 0.0)
            for c in range(4):
                nc.tensor.matmul(out2_all[:, 512 * c:512 * (c + 1)], z65[:],
                                 zrow[:], start=True, stop=False)

            zX = S.tile([HD + 1, N], FP)  # gated attn out^T + denom row

            Pks = [None] * NT

            def emit_mm1(tk):
                ksl = slice(128 * tk, 128 * (tk + 1))
                groups = []
                nq_near = min(3, NT - tk)
                groups.append(([0, 1, 2][:nq_near], tk, nq_near))
                nq34 = max(0, min(2, NT - tk - 3))
                if nq34:
                    groups.append(([3, 4][:nq34], tk + 3, nq34))
                if tk + 8 < NT:
                    groups.append(([6, None, 8], tk + 6, 3))
                elif tk + 6 < NT:
                    groups.append(([6], tk + 6, 1))
                if tk + 12 < NT:
                    groups.append(([12], tk + 12, 1))
                Pk = PK.tile([128, NR, 128], F16, tag="Pk")
                Pks[tk] = Pk
                for ms, q_lo, nq in groups:
                    qsl = slice(128 * q_lo, 128 * (q_lo + nq))
                    psS = PS3.tile([128, 384], FP, tag="s")
                    nc.tensor.matmul(psS[:, 0:128 * nq], kT[:, ksl],
                                     qT[:, qsl], start=True, stop=True)
                    expS = W.tile([128, 384], F16, tag="expS")
                    nc.scalar.activation(expS[:, 0:128 * nq], psS[:, 0:128 * nq],
                                         mybir.ActivationFunctionType.Exp)
                    ris = [R_DEPTHS.index(m) for m in ms if m is not None]
                    if ms == [6, None, 8]:
                        b = expS[:]
                        srcap = bass.AP(tensor=b.tensor, offset=b.offset,
                                        ap=[b.ap[0], [256, 2], [1, 128]])
                        nc.gpsimd.tensor_mul(Pk[:, ris[0]:ris[0] + 2, :],
                                             srcap, mws[:, ris[0]:ris[0] + 2, :])
                    elif ris[0] >= 5:
                        nc.gpsimd.tensor_mul(
                            Pk[:, ris[0]:ris[0] + len(ris), :],
                            expS[:, 0:128 * len(ris)],
                            mws[:, ris[0]:ris[0] + len(ris), :])
                    else:
                        nc.vector.tensor_mul(
                            Pk[:, ris[0]:ris[0] + len(ris), :],
                            expS[:, 0:128 * len(ris)],
                            mws[:, ris[0]:ris[0] + len(ris), :])

            def emit_mm2(tk):
                Pk = Pks[tk]
                nq1 = min(4, NT - tk)
                nc.tensor.matmul(
                    out2_all[:, 128 * tk:128 * (tk + nq1)],
                    Vn[:, tk, :], Pk[:, 0:nq1, :],
                    start=False, stop=(tk == NT - 1), skip_group_check=True)
                for ri, m in ((4, 4), (5, 6), (6, 8), (7, 12)):
                    if tk + m < NT:
                        nc.tensor.matmul(
                            out2_all[:, 128 * (tk + m):128 * (tk + m + 1)],
                            Vn[:, tk, :], Pk[:, ri, :],
                            start=False, stop=False, skip_group_check=True)

            def emit_epilogue(c):
                csl = slice(512 * c, 512 * (c + 1))
                nc.vector.tensor_mul(zX[:, csl], out2_all[:, csl], gT[:, csl])

            for tk in range(NT):
                emit_mm1(tk)
                if tk >= 1:
                    emit_mm2(tk - 1)
                if tk >= 5 and (tk - 5) % 4 == 0:
                    emit_epilogue((tk - 5) // 4)
            emit_mm2(NT - 1)
            emit_epilogue(2)
            emit_epilogue(3)

            # ---------- AllToAll exchange ----------
            bin_ = DR.tile([NC, HD + 1, NB], FP)
            bout_ = DR.tile([NC, HD + 1, NB], FP)
            for j in range(NC):
                nc.sync.dma_start(out=bin_[j],
                                  in_=zX[:, NB * j:NB * (j + 1)])
            nc.gpsimd.collective_compute(
                "AllToAll", mybir.AluOpType.bypass,
                replica_groups=[list(range(NC))],
                ins=[bin_[:].opt()], outs=[bout_[:].opt()],
            )

            # ---------- stage 3: normalize + out projection ----------
            # pair-stack heads on partitions: zr2[c, p, n] = z^T[(2p)*64+c] rows
            SRC = NC * (HD + 1) * NB   # element stride between sources in bout_
            bo = bout_[:]
            zr2 = S.tile([128, 4, NB], FP)
            for par in range(2):
                inap = bass.AP(tensor=bo.tensor,
                               offset=bo.offset + par * (HD + 1) * NB,
                               ap=[[NB, 64], [2 * (HD + 1) * NB, 4], [1, NB]])
                nc.sync.dma_start(out=zr2[64 * par:64 * (par + 1), :, :], in_=inap)
            rb2 = S.tile([128, 4, NB], FP)
            for par in range(2):
                inap = bass.AP(tensor=bo.tensor,
                               offset=bo.offset + HD * NB + par * (HD + 1) * NB,
                               ap=[[0, 64], [2 * (HD + 1) * NB, 4], [1, NB]])
                nc.sync.dma_start(out=rb2[64 * par:64 * (par + 1), :, :], in_=inap)
            rr2 = S.tile([128, 4, NB], FP)
            rscr = S.tile([128, 4, NB], FP)
            nc.vector.reciprocal_approx_accurate(rr2[:], rb2[:], rscr[:])
            zn2 = S.tile([128, 4, NB], F16)
            nc.vector.tensor_mul(zn2[:], zr2[:], rr2[:])

            for nt in range(NB // 128):
                nsl = slice(128 * nt, 128 * (nt + 1))
                psY = PS3.tile([128, D], FP, tag="s")
                for p in range(4):
                    nc.tensor.matmul(psY[:], zn2[:, p, nsl], wos[:, p, :],
                                     start=(p == 0), stop=False)
                nc.tensor.matmul(psY[:], ones_r[:, 0:128], bos[:],
                                 start=False, stop=True)
                ysb = W.tile([128, D], FP, tag="ysb")
                nc.scalar.copy(ysb[:], psY[:])
                nc.sync.dma_start(out=yout[nsl, :], in_=ysb[:])

    nc.compile()
    return nc


def _prep_inputs(x, W_qkv, b_qkv, W_out, b_out, W_gate, b_gate,
                 pos_bias, scale_embed, if_gain, disp_amp):
    assert not np.any(np.asarray(scale_embed)), \
        "kernel fast path requires scale_embed == 0"
    xTn = np.ascontiguousarray(np.asarray(x)[0].T.astype(np.float32))  # [D, N]
    W_qkv = np.asarray(W_qkv, dtype=np.float32)
    b_qkv = np.asarray(b_qkv, dtype=np.float32)
    W_gate = np.asarray(W_gate, dtype=np.float32)
    b_gate = np.asarray(b_gate, dtype=np.float32)
    W_out = np.asarray(W_out, dtype=np.float32)
    b_out = np.asarray(b_out, dtype=np.float32)
    pos_bias = np.asarray(pos_bias, dtype=np.float32)
    if_gain = np.asarray(if_gain, dtype=np.float32)
    disp_amp = np.asarray(disp_amp, dtype=np.float32)

    scl = 1.0 / math.sqrt(HD)
    wout16 = np.ascontiguousarray(
        W_out.reshape(4, 128, D).transpose(1, 0, 2)).astype(np.float16)  # [128,4,D]
    xT16 = xTn.astype(np.float16)

    in_maps = []
    for h in range(NC):
        qs = slice(HD * h, HD * (h + 1))
        ks = slice(D + HD * h, D + HD * (h + 1))
        vs = slice(2 * D + HD * h, 2 * D + HD * (h + 1))
        wq = W_qkv[:, qs] * scl
        wk = W_qkv[:, ks]
        wv = W_qkv[:, vs] * if_gain[h]
        wg = W_gate[:, qs]
        bq = b_qkv[qs] * scl
        bk = b_qkv[ks]
        bv = b_qkv[vs] * if_gain[h]
        bg = b_gate[qs]
        eff_pb_h = pos_bias[:, h] + DISP_COS_KERNEL[:, h] * disp_amp[h]
        in_maps.append({
            "xT": xT16,
            "wA": np.ascontiguousarray(
                np.concatenate([wq, wk], axis=1)).astype(np.float16),
            "wB": np.ascontiguousarray(
                np.concatenate([wv, wg], axis=1)).astype(np.float16),
            "bA": np.ascontiguousarray(np.concatenate([bq, bk])),
            "bB": np.ascontiguousarray(np.concatenate([bv, bg])),
            "maskW": _build_masks(eff_pb_h).astype(np.float16),
            "woutS": wout16,
            "bout": b_out.astype(np.float16),
        })
    return in_maps


def kernel(**inputs) -> np.ndarray:
    if "nc" not in _cache:
        _cache["nc"] = _build_module()
    nc = _cache["nc"]
    in_maps = _prep_inputs(**inputs)
    res = run_bass_kernel_spmd(nc, in_maps, core_ids=list(range(NC)))
    y = np.concatenate([res.results[c]["y"] for c in range(NC)], axis=0)
    return y.reshape(B, N, D)


# revision 22
# speedup vs baseline: 2.2232x; 1.0324x over previous
"""Trainium2 Bass kernel for nn_DSQGAttentionQW (sparse offset attention).

Sharding: head-tensor-parallel attention (8 heads -> 8 cores) + AllToAll
re-shard to sequence-parallel for the output projection. Single NEFF launch.
"""
import math
import numpy as np

import concourse.bacc as bacc
import concourse.bass as bass
import concourse.tile as tile
import concourse.mybir as mybir
import concourse.masks as masks
from concourse.bass_utils import run_bass_kernel_spmd

# ---- problem constants (must match reference.py) ----
_DENSE_LOCAL_W = 32
_DYADIC = [48, 64, 96, 128, 192, 256, 384, 512, 768, 1024, 1536, 2048, 3072, 4096]
OFFSETS = np.array(
    sorted(set(range(0, _DENSE_LOCAL_W + 1)) | set(_DYADIC)), dtype=np.int32
)  # [47]
NUM_OFFSETS = len(OFFSETS)
H = 8
_LOG_MAX = math.log(1.0 + 4096.0)
_HEAD_OMEGAS = [0.0, 0.0, 1 * math.pi / _LOG_MAX, 1 * math.pi / _LOG_MAX,
                4 * math.pi / _LOG_MAX, 4 * math.pi / _LOG_MAX,
                6 * math.pi / _LOG_MAX, 6 * math.pi / _LOG_MAX]
_log_d = np.log(1.0 + OFFSETS.astype(np.float64))
DISP_COS_KERNEL = np.zeros((NUM_OFFSETS, H), dtype=np.float32)
for _h, _om in enumerate(_HEAD_OMEGAS):
    if _om > 0.0:
        DISP_COS_KERNEL[:, _h] = np.cos(_om * _log_d)

B, N, D = 1, 2048, 512
HD = D // H
NC = 8
NB = N // NC            # 256: per-core output row block
NT = N // 128           # 16 q-tiles of 128
# Effective k-tile depths m (delta in (128(m-1), 128m]) that can be causal for
# N=2048: depths 16/24/32 (delta >= 2048) are never valid.
R_DEPTHS = [0, 1, 2, 3, 4, 6, 8, 12]
NR = len(R_DEPTHS)

FP = mybir.dt.float32
FR = mybir.dt.float32r
F16 = mybir.dt.float16

_cache = {}


def _build_masks(eff_pb_h: np.ndarray) -> np.ndarray:
    """maskW[ri, kp, i] = exp(eff_pb[offset_idx(delta)]) if delta valid else 0,
    with delta = i - kp + 128*m for depth m = R_DEPTHS[ri]."""
    off_idx = {int(d): i for i, d in enumerate(OFFSETS)}
    kp = np.arange(128)[None, :, None]
    i = np.arange(128)[None, None, :]
    m = np.array(R_DEPTHS)[:, None, None]
    delta = i - kp + 128 * m  # [NR, 128, 128]
    w = np.zeros((NR, 128, 128), dtype=np.float32)
    for d, oi in off_idx.items():
        sel = delta == d
        if sel.any():
            w[sel] = math.exp(float(eff_pb_h[oi]))
    return w


def _r(ap):
    return ap.bitcast(FR)


def _build_module():
    nc = bacc.Bacc("TRN2", target_bir_lowering=False, debug=False, num_devices=NC)

    xT = nc.dram_tensor("xT", [D, N], F16, kind="ExternalInput").ap()
    wA = nc.dram_tensor("wA", [D, 128], F16, kind="ExternalInput").ap()   # [Wq|Wk]
    wB = nc.dram_tensor("wB", [D, 128], F16, kind="ExternalInput").ap()   # [Wv|Wg]
    bA = nc.dram_tensor("bA", [128], FP, kind="ExternalInput").ap()
    bB = nc.dram_tensor("bB", [128], FP, kind="ExternalInput").ap()
    maskW = nc.dram_tensor("maskW", [NR, 128, 128], F16, kind="ExternalInput").ap()
    woutS = nc.dram_tensor("woutS", [128, 4, D], F16, kind="ExternalInput").ap()
    bout = nc.dram_tensor("bout", [D], F16, kind="ExternalInput").ap()
    yout = nc.dram_tensor("y", [NB, D], FP, kind="ExternalOutput").ap()

    with tile.TileContext(nc) as tc:
        with (
            tc.tile_pool(name="singles", bufs=1) as S,
            tc.tile_pool(name="work", bufs=3) as W,
            tc.tile_pool(name="pk", bufs=3) as PK,
            tc.tile_pool(name="ps", bufs=1, space="PSUM") as PS,
            tc.tile_pool(name="ps3", bufs=3, space="PSUM") as PS3,
            tc.tile_pool(name="pso", bufs=1, space="PSUM") as PSO,
            tc.tile_pool(name="dram", bufs=1, space="DRAM") as DR,
        ):
            # ---------- constants / loads ----------
            ident = S.tile([128, 128], F16)
            masks.make_identity(nc, ident[:])
            ones_r = S.tile([1, 128], F16)
            nc.vector.memset(ones_r[:], 1.0)

            wAs = S.tile([128, 4, 128], F16)
            nc.sync.dma_start(out=wAs[:], in_=wA.rearrange("(ct p) o -> p ct o", p=128))
            wBs = S.tile([128, 4, 128], F16)
            nc.sync.dma_start(out=wBs[:], in_=wB.rearrange("(ct p) o -> p ct o", p=128))
            bAs = S.tile([128, 1], FP)
            nc.sync.dma_start(out=bAs[:], in_=bA[:, None])
            bBs = S.tile([128, 1], FP)
            nc.sync.dma_start(out=bBs[:], in_=bB[:, None])

            xs = S.tile([128, 4, N], F16)
            xT_r = xT.rearrange("(ct p) n -> p ct n", p=128)
            for nch in range(4):
                for ct in range(4):
                    nsl = slice(512 * nch, 512 * (nch + 1))
                    nc.sync.dma_start(out=xs[:, ct, nsl], in_=xT_r[:, ct, nsl])

            mws = S.tile([128, NR, 128], F16)
            nc.sync.dma_start(out=mws[:], in_=maskW.rearrange("r kp i -> kp r i"))
            wos = S.tile([128, 4, D], F16)
            nc.sync.dma_start(out=wos[:], in_=woutS[:])
            bos = S.tile([1, D], F16)
            nc.sync.dma_start(out=bos[:], in_=bout[None, :])

            # ---------- MM-A: qT / kT / vT / gT (fp32r, N=512) ----------
            qT = S.tile([64, N], F16)       # pre-scaled by 1/sqrt(HD)
            kT = S.tile([64, N], F16)
            vT = S.tile([64, N], F16)       # if_gain folded
            gT = S.tile([HD + 1, N], FP)   # sigmoid gate; row 64 = 1.0 (denom)
            nc.vector.memset(gT[HD:HD + 1, :], 1.0)
            Vn = S.tile([128, NT, HD + 1], F16)
            nc.vector.memset(Vn[:, :, HD:HD + 1], 1.0)

            def emit_transpose(t):
                psT = PS3.tile([128, 64], F16, tag="s")
                nc.tensor.transpose(psT[:], vT[:, 128 * t:128 * (t + 1)],
                                    ident[0:64, 0:64])
                nc.vector.tensor_copy(Vn[:, t, 0:HD], psT[:])

            for nch in range(4):
                nsl = slice(512 * nch, 512 * (nch + 1))
                psA = PS.tile([128, 512], FP, tag="mma")
                psB = PS3.tile([128, 512], FP, tag="s")
                for ct in range(4):
                    nc.tensor.matmul(psA[:], wAs[:, ct, :], xs[:, ct, nsl],
                                     start=(ct == 0), stop=(ct == 3))
                for ct in range(4):
                    nc.tensor.matmul(psB[:], wBs[:, ct, :], xs[:, ct, nsl],
                                     start=(ct == 0), stop=(ct == 3))
                # biased copies: q,k,v on DVE; gate sigmoid on ACT
                nc.vector.tensor_scalar_add(qT[:, nsl], psA[0:64, :], bAs[0:64])
                nc.vector.tensor_scalar_add(kT[:, nsl], psA[64:128, :], bAs[64:128])
                nc.scalar.activation(vT[:, nsl], psB[0:64, :],
                                     mybir.ActivationFunctionType.Identity,
                                     bias=bBs[0:64], scale=1.0)
                nc.scalar.activation(gT[0:HD, nsl], psB[64:128, :],
                                     mybir.ActivationFunctionType.Sigmoid,
                                     bias=bBs[64:128], scale=1.0)
                if nch >= 1:
                    for t in range(4 * (nch - 1), 4 * nch):
                        emit_transpose(t)


            for t in range(12, 16):
                emit_transpose(t)

            # ---------- attention (k-tile-major, fp16) ----------
            # out2_all: [65, N] PSUM accumulator (4 banks). Zero-init via K=1
            # start=True matmuls so subsequent MMs accumulate via has_written.
            out2_all = PSO.tile([HD + 1, N], FP)
            zrow = S.tile([1, 512], F16)
            nc.vector.memset(zrow[:], 0.0)
            z65 = S.tile([1, HD + 1], F16)
            nc.vector.memset(z65[:], 0.0)
            for c in range(4):
                nc.tensor.matmul(out2_all[:, 512 * c:512 * (c + 1)], z65[:],
                                 zrow[:], start=True, stop=False)

            zX = S.tile([HD + 1, N], FP)  # gated attn out^T + denom row

            Pks = [None] * NT

            def emit_mm1(tk):
                ksl = slice(128 * tk, 128 * (tk + 1))
                groups = []
                nq_near = min(3, NT - tk)
                groups.append(([0, 1, 2][:nq_near], tk, nq_near))
                nq34 = max(0, min(2, NT - tk - 3))
                if nq34:
                    groups.append(([3, 4][:nq34], tk + 3, nq34))
                if tk + 8 < NT:
                    groups.append(([6, None, 8], tk + 6, 3))
                elif tk + 6 < NT:
                    groups.append(([6], tk + 6, 1))
                if tk + 12 < NT:
                    groups.append(([12], tk + 12, 1))
                Pk = PK.tile([128, NR, 128], F16, tag="Pk")
                Pks[tk] = Pk
                for ms, q_lo, nq in groups:
                    qsl = slice(128 * q_lo, 128 * (q_lo + nq))
                    psS = PS3.tile([128, 384], FP, tag="s")
                    nc.tensor.matmul(psS[:, 0:128 * nq], kT[:, ksl],
                                     qT[:, qsl], start=True, stop=True)
                    expS = W.tile([128, 384], F16, tag="expS")
                    nc.scalar.activation(expS[:, 0:128 * nq], psS[:, 0:128 * nq],
                                         mybir.ActivationFunctionType.Exp)
                    ris = [R_DEPTHS.index(m) for m in ms if m is not None]
                    if ms == [6, None, 8]:
                        b = expS[:]
                        srcap = bass.AP(tensor=b.tensor, offset=b.offset,
                                        ap=[b.ap[0], [256, 2], [1, 128]])
                        nc.gpsimd.tensor_mul(Pk[:, ris[0]:ris[0] + 2, :],
                                             srcap, mws[:, ris[0]:ris[0] + 2, :])
                    elif ris[0] >= 5:
                        nc.gpsimd.tensor_mul(
                            Pk[:, ris[0]:ris[0] + len(ris), :],
                            expS[:, 0:128 * len(ris)],
                            mws[:, ris[0]:ris[0] + len(ris), :])
                    else:
                        nc.vector.tensor_mul(
                            Pk[:, ris[0]:ris[0] + len(ris), :],
                            expS[:, 0:128 * len(ris)],
                            mws[:, ris[0]:ris[0] + len(ris), :])

            def emit_mm2(tk):
                Pk = Pks[tk]
                nq1 = min(4, NT - tk)
                nc.tensor.matmul(
                    out2_all[:, 128 * tk:128 * (tk + nq1)],
                    Vn[:, tk, :], Pk[:, 0:nq1, :],
                    start=False, stop=(tk == NT - 1), skip_group_check=True)
                for ri, m in ((4, 4), (5, 6), (6, 8), (7, 12)):
                    if tk + m < NT:
                        nc.tensor.matmul(
                            out2_all[:, 128 * (tk + m):128 * (tk + m + 1)],
                            Vn[:, tk, :], Pk[:, ri, :],
                            start=False, stop=False, skip_group_check=True)

            def emit_epilogue(c):
                csl = slice(512 * c, 512 * (c + 1))
                nc.vector.tensor_mul(zX[:, csl], out2_all[:, csl], gT[:, csl])

            for tk in range(NT):
                emit_mm1(tk)
                if tk >= 1:
                    emit_mm2(tk - 1)
                if tk >= 5 and (tk - 5) % 4 == 0:
                    emit_epilogue((tk - 5) // 4)
            emit_mm2(NT - 1)
            emit_epilogue(2)
            emit_epilogue(3)

            # ---------- AllToAll exchange ----------
            bin_ = DR.tile([NC, HD + 1, NB], FP)
            bout_ = DR.tile([NC, HD + 1, NB], FP)
            for j in range(NC):
                nc.sync.dma_start(out=bin_[j],
                                  in_=zX[:, NB * j:NB * (j + 1)])
            nc.gpsimd.collective_compute(
                "AllToAll", mybir.AluOpType.bypass,
                replica_groups=[list(range(NC))],
                ins=[bin_[:].opt()], outs=[bout_[:].opt()],
            )

            # ---------- stage 3: normalize + out projection ----------
            # pair-stack heads on partitions: zr2[c, p, n] = z^T[(2p)*64+c] rows
            SRC = NC * (HD + 1) * NB   # element stride between sources in bout_
            bo = bout_[:]
            zr2 = S.tile([128, 4, NB], FP)
            for par in range(2):
                inap = bass.AP(tensor=bo.tensor,
                               offset=bo.offset + par * (HD + 1) * NB,
                               ap=[[NB, 64], [2 * (HD + 1) * NB, 4], [1, NB]])
                nc.sync.dma_start(out=zr2[64 * par:64 * (par + 1), :, :], in_=inap)
            rb2 = S.tile([128, 4, NB], FP)
            for par in range(2):
                inap = bass.AP(tensor=bo.tensor,
                               offset=bo.offset + HD * NB + par * (HD + 1) * NB,
                               ap=[[0, 64], [2 * (HD + 1) * NB, 4], [1, NB]])
                nc.sync.dma_start(out=rb2[64 * par:64 * (par + 1), :, :], in_=inap)
            rr2 = S.tile([128, 4, NB], FP)
            rscr = S.tile([128, 4, NB], FP)
            nc.vector.reciprocal_approx_accurate(rr2[:], rb2[:], rscr[:])
            zn2 = S.tile([128, 4, NB], F16)
            nc.vector.tensor_mul(zn2[:], zr2[:], rr2[:])

            for nt in range(NB // 128):
                nsl = slice(128 * nt, 128 * (nt + 1))
                psY = PS3.tile([128, D], FP, tag="s")
                for p in range(4):
                    nc.tensor.matmul(psY[:], zn2[:, p, nsl], wos[:, p, :],
                                     start=(p == 0), stop=False)
                nc.tensor.matmul(psY[:], ones_r[:, 0:128], bos[:],
                                 start=False, stop=True)
                ysb = W.tile([128, D], FP, tag="ysb")
                nc.scalar.copy(ysb[:], psY[:])
                nc.sync.dma_start(out=yout[nsl, :], in_=ysb[:])

    nc.compile()
    return nc


def _prep_inputs(x, W_qkv, b_qkv, W_out, b_out, W_gate, b_gate,
                 pos_bias, scale_embed, if_gain, disp_amp):
    assert not np.any(np.asarray(scale_embed)), \
        "kernel fast path requires scale_embed == 0"
    xTn = np.ascontiguousarray(np.asarray(x)[0].T.astype(np.float32))  # [D, N]
    W_qkv = np.asarray(W_qkv, dtype=np.float32)
    b_qkv = np.asarray(b_qkv, dtype=np.float32)
    W_gate = np.asarray(W_gate, dtype=np.float32)
    b_gate = np.asarray(b_gate, dtype=np.float32)
    W_out = np.asarray(W_out, dtype=np.float32)
    b_out = np.asarray(b_out, dtype=np.float32)
    pos_bias = np.asarray(pos_bias, dtype=np.float32)
    if_gain = np.asarray(if_gain, dtype=np.float32)
    disp_amp = np.asarray(disp_amp, dtype=np.float32)

    scl = 1.0 / math.sqrt(HD)
    wout16 = np.ascontiguousarray(
        W_out.reshape(4, 128, D).transpose(1, 0, 2)).astype(np.float16)  # [128,4,D]
    xT16 = xTn.astype(np.float16)

    in_maps = []
    for h in range(NC):
        qs = slice(HD * h, HD * (h + 1))
        ks = slice(D + HD * h, D + HD * (h + 1))
        vs = slice(2 * D + HD * h, 2 * D + HD * (h + 1))
        wq = W_qkv[:, qs] * scl
        wk = W_qkv[:, ks]
        wv = W_qkv[:, vs] * if_gain[h]
        wg = W_gate[:, qs]
        bq = b_qkv[qs] * scl
        bk = b_qkv[ks]
        bv = b_qkv[vs] * if_gain[h]
        bg = b_gate[qs]
        eff_pb_h = pos_bias[:, h] + DISP_COS_KERNEL[:, h] * disp_amp[h]
        in_maps.append({
            "xT": xT16,
            "wA": np.ascontiguousarray(
                np.concatenate([wq, wk], axis=1)).astype(np.float16),
            "wB": np.ascontiguousarray(
                np.concatenate([wv, wg], axis=1)).astype(np.float16),
            "bA": np.ascontiguousarray(np.concatenate([bq, bk])),
            "bB": np.ascontiguousarray(np.concatenate([bv, bg])),
            "maskW": _build_masks(eff_pb_h).astype(np.float16),
            "woutS": wout16,
            "bout": b_out.astype(np.float16),
        })
    return in_maps


def kernel(**inputs) -> np.ndarray:
    if "nc" not in _cache:
        _cache["nc"] = _build_module()
    nc = _cache["nc"]
    in_maps = _prep_inputs(**inputs)
    res = run_bass_kernel_spmd(nc, in_maps, core_ids=list(range(NC)))
    y = np.concatenate([res.results[c]["y"] for c in range(NC)], axis=0)
    return y.reshape(B, N, D)


# revision 23
# speedup vs baseline: 2.2972x; 1.0333x over previous
"""Trainium2 Bass kernel for nn_DSQGAttentionQW (sparse offset attention).

Sharding: head-tensor-parallel attention (8 heads -> 8 cores) + AllToAll
re-shard to sequence-parallel for the output projection. Single NEFF launch.
"""
import math
import numpy as np

import concourse.bacc as bacc
import concourse.bass as bass
import concourse.tile as tile
import concourse.mybir as mybir
import concourse.masks as masks
from concourse.bass_utils import run_bass_kernel_spmd

# ---- problem constants (must match reference.py) ----
_DENSE_LOCAL_W = 32
_DYADIC = [48, 64, 96, 128, 192, 256, 384, 512, 768, 1024, 1536, 2048, 3072, 4096]
OFFSETS = np.array(
    sorted(set(range(0, _DENSE_LOCAL_W + 1)) | set(_DYADIC)), dtype=np.int32
)  # [47]
NUM_OFFSETS = len(OFFSETS)
H = 8
_LOG_MAX = math.log(1.0 + 4096.0)
_HEAD_OMEGAS = [0.0, 0.0, 1 * math.pi / _LOG_MAX, 1 * math.pi / _LOG_MAX,
                4 * math.pi / _LOG_MAX, 4 * math.pi / _LOG_MAX,
                6 * math.pi / _LOG_MAX, 6 * math.pi / _LOG_MAX]
_log_d = np.log(1.0 + OFFSETS.astype(np.float64))
DISP_COS_KERNEL = np.zeros((NUM_OFFSETS, H), dtype=np.float32)
for _h, _om in enumerate(_HEAD_OMEGAS):
    if _om > 0.0:
        DISP_COS_KERNEL[:, _h] = np.cos(_om * _log_d)

B, N, D = 1, 2048, 512
HD = D // H
NC = 8
NB = N // NC            # 256: per-core output row block
NT = N // 128           # 16 q-tiles of 128
# Effective k-tile depths m (delta in (128(m-1), 128m]) that can be causal for
# N=2048: depths 16/24/32 (delta >= 2048) are never valid.
R_DEPTHS = [0, 1, 2, 3, 4, 6, 8, 12]
NR = len(R_DEPTHS)

FP = mybir.dt.float32
FR = mybir.dt.float32r
F16 = mybir.dt.float16

_cache = {}


def _build_masks(eff_pb_h: np.ndarray) -> np.ndarray:
    """maskW[ri, kp, i] = exp(eff_pb[offset_idx(delta)]) if delta valid else 0,
    with delta = i - kp + 128*m for depth m = R_DEPTHS[ri]."""
    off_idx = {int(d): i for i, d in enumerate(OFFSETS)}
    kp = np.arange(128)[None, :, None]
    i = np.arange(128)[None, None, :]
    m = np.array(R_DEPTHS)[:, None, None]
    delta = i - kp + 128 * m  # [NR, 128, 128]
    w = np.zeros((NR, 128, 128), dtype=np.float32)
    for d, oi in off_idx.items():
        sel = delta == d
        if sel.any():
            w[sel] = math.exp(float(eff_pb_h[oi]))
    return w


def _r(ap):
    return ap.bitcast(FR)


def _build_module():
    nc = bacc.Bacc("TRN2", target_bir_lowering=False, debug=False, num_devices=NC)

    xT = nc.dram_tensor("xT", [D, N], F16, kind="ExternalInput").ap()
    wA = nc.dram_tensor("wA", [D, 128], F16, kind="ExternalInput").ap()   # [Wq|Wk]
    wB = nc.dram_tensor("wB", [D, 128], F16, kind="ExternalInput").ap()   # [Wv|Wg]
    bA = nc.dram_tensor("bA", [128], FP, kind="ExternalInput").ap()
    bB = nc.dram_tensor("bB", [128], FP, kind="ExternalInput").ap()
    maskW = nc.dram_tensor("maskW", [NR, 128, 128], F16, kind="ExternalInput").ap()
    woutS = nc.dram_tensor("woutS", [128, 4, D], F16, kind="ExternalInput").ap()
    bout = nc.dram_tensor("bout", [D], F16, kind="ExternalInput").ap()
    yout = nc.dram_tensor("y", [NB, D], FP, kind="ExternalOutput").ap()

    with tile.TileContext(nc) as tc:
        with (
            tc.tile_pool(name="singles", bufs=1) as S,
            tc.tile_pool(name="work", bufs=3) as W,
            tc.tile_pool(name="pk", bufs=3) as PK,
            tc.tile_pool(name="ps", bufs=1, space="PSUM") as PS,
            tc.tile_pool(name="ps3", bufs=3, space="PSUM") as PS3,
            tc.tile_pool(name="pso", bufs=1, space="PSUM") as PSO,
            tc.tile_pool(name="dram", bufs=1, space="DRAM") as DR,
        ):
            # ---------- PE warm-up (HAM) during initial DMA window ----------
            wconst = S.tile([128, 512], F16)
            nc.vector.memset(wconst[:], 1.0)
            psW = PS3.tile([128, 512], FP, tag="s")
            for _ in range(16):
                nc.tensor.matmul(psW[:], wconst[:, 0:128], wconst[:],
                                 start=True, stop=True)

            # ---------- constants / loads ----------
            ident = S.tile([128, 128], F16)
            masks.make_identity(nc, ident[:])
            ones_r = S.tile([1, 128], F16)
            nc.vector.memset(ones_r[:], 1.0)

            wAs = S.tile([128, 4, 128], F16)
            nc.sync.dma_start(out=wAs[:], in_=wA.rearrange("(ct p) o -> p ct o", p=128))
            wBs = S.tile([128, 4, 128], F16)
            nc.sync.dma_start(out=wBs[:], in_=wB.rearrange("(ct p) o -> p ct o", p=128))
            bAs = S.tile([128, 1], FP)
            nc.sync.dma_start(out=bAs[:], in_=bA[:, None])
            bBs = S.tile([128, 1], FP)
            nc.sync.dma_start(out=bBs[:], in_=bB[:, None])

            xs = S.tile([128, 4, N], F16)
            xT_r = xT.rearrange("(ct p) n -> p ct n", p=128)
            for nch in range(4):
                for ct in range(4):
                    nsl = slice(512 * nch, 512 * (nch + 1))
                    nc.sync.dma_start(out=xs[:, ct, nsl], in_=xT_r[:, ct, nsl])

            mws = S.tile([128, NR, 128], F16)
            nc.sync.dma_start(out=mws[:], in_=maskW.rearrange("r kp i -> kp r i"))
            wos = S.tile([128, 4, D], F16)
            nc.sync.dma_start(out=wos[:], in_=woutS[:])
            bos = S.tile([1, D], F16)
            nc.sync.dma_start(out=bos[:], in_=bout[None, :])

            # ---------- MM-A: qT / kT / vT / gT (fp32r, N=512) ----------
            qT = S.tile([64, N], F16)       # pre-scaled by 1/sqrt(HD)
            kT = S.tile([64, N], F16)
            vT = S.tile([64, N], F16)       # if_gain folded
            gT = S.tile([HD + 1, N], FP)   # sigmoid gate; row 64 = 1.0 (denom)
            nc.vector.memset(gT[HD:HD + 1, :], 1.0)
            Vn = S.tile([128, NT, HD + 1], F16)
            nc.vector.memset(Vn[:, :, HD:HD + 1], 1.0)

            def emit_transpose(t):
                psT = PS3.tile([128, 64], F16, tag="s")
                nc.tensor.transpose(psT[:], vT[:, 128 * t:128 * (t + 1)],
                                    ident[0:64, 0:64])
                nc.vector.tensor_copy(Vn[:, t, 0:HD], psT[:])

            for nch in range(4):
                nsl = slice(512 * nch, 512 * (nch + 1))
                psA = PS.tile([128, 512], FP, tag="mma")
                psB = PS3.tile([128, 512], FP, tag="s")
                for ct in range(4):
                    nc.tensor.matmul(psA[:], wAs[:, ct, :], xs[:, ct, nsl],
                                     start=(ct == 0), stop=(ct == 3))
                for ct in range(4):
                    nc.tensor.matmul(psB[:], wBs[:, ct, :], xs[:, ct, nsl],
                                     start=(ct == 0), stop=(ct == 3))
                # biased copies: q,k,v on DVE; gate sigmoid on ACT
                nc.vector.tensor_scalar_add(qT[:, nsl], psA[0:64, :], bAs[0:64])
                nc.vector.tensor_scalar_add(kT[:, nsl], psA[64:128, :], bAs[64:128])
                nc.scalar.activation(vT[:, nsl], psB[0:64, :],
                                     mybir.ActivationFunctionType.Identity,
                                     bias=bBs[0:64], scale=1.0)
                nc.scalar.activation(gT[0:HD, nsl], psB[64:128, :],
                                     mybir.ActivationFunctionType.Sigmoid,
                                     bias=bBs[64:128], scale=1.0)
                if nch >= 1:
                    for t in range(4 * (nch - 1), 4 * nch):
                        emit_transpose(t)


            for t in range(12, 16):
                emit_transpose(t)

            # ---------- attention (k-tile-major, fp16) ----------
            # out2_all: [65, N] PSUM accumulator (4 banks). Zero-init via K=1
            # start=True matmuls so subsequent MMs accumulate via has_written.
            out2_all = PSO.tile([HD + 1, N], FP)
            zrow = S.tile([1, 512], F16)
            nc.vector.memset(zrow[:], 0.0)
            z65 = S.tile([1, HD + 1], F16)
            nc.vector.memset(z65[:], 0.0)
            for c in range(4):
                nc.tensor.matmul(out2_all[:, 512 * c:512 * (c + 1)], z65[:],
                                 zrow[:], start=True, stop=False)

            zX = S.tile([HD + 1, N], FP)  # gated attn out^T + denom row

            Pks = [None] * NT

            def emit_mm1(tk):
                ksl = slice(128 * tk, 128 * (tk + 1))
                groups = []
                nq_near = min(3, NT - tk)
                groups.append(([0, 1, 2][:nq_near], tk, nq_near))
                nq34 = max(0, min(2, NT - tk - 3))
                if nq34:
                    groups.append(([3, 4][:nq34], tk + 3, nq34))
                if tk + 8 < NT:
                    groups.append(([6, None, 8], tk + 6, 3))
                elif tk + 6 < NT:
                    groups.append(([6], tk + 6, 1))
                if tk + 12 < NT:
                    groups.append(([12], tk + 12, 1))
                Pk = PK.tile([128, NR, 128], F16, tag="Pk")
                Pks[tk] = Pk
                for ms, q_lo, nq in groups:
                    qsl = slice(128 * q_lo, 128 * (q_lo + nq))
                    psS = PS3.tile([128, 384], FP, tag="s")
                    nc.tensor.matmul(psS[:, 0:128 * nq], kT[:, ksl],
                                     qT[:, qsl], start=True, stop=True)
                    expS = W.tile([128, 384], F16, tag="expS")
                    nc.scalar.activation(expS[:, 0:128 * nq], psS[:, 0:128 * nq],
                                         mybir.ActivationFunctionType.Exp)
                    ris = [R_DEPTHS.index(m) for m in ms if m is not None]
                    if ms == [6, None, 8]:
                        b = expS[:]
                        srcap = bass.AP(tensor=b.tensor, offset=b.offset,
                                        ap=[b.ap[0], [256, 2], [1, 128]])
                        nc.gpsimd.tensor_mul(Pk[:, ris[0]:ris[0] + 2, :],
                                             srcap, mws[:, ris[0]:ris[0] + 2, :])
                    elif ris[0] >= 5:
                        nc.gpsimd.tensor_mul(
                            Pk[:, ris[0]:ris[0] + len(ris), :],
                            expS[:, 0:128 * len(ris)],
                            mws[:, ris[0]:ris[0] + len(ris), :])
                    else:
                        nc.vector.tensor_mul(
                            Pk[:, ris[0]:ris[0] + len(ris), :],
                            expS[:, 0:128 * len(ris)],
                            mws[:, ris[0]:ris[0] + len(ris), :])

            def emit_mm2(tk):
                Pk = Pks[tk]
                nq1 = min(4, NT - tk)
                nc.tensor.matmul(
                    out2_all[:, 128 * tk:128 * (tk + nq1)],
                    Vn[:, tk, :], Pk[:, 0:nq1, :],
                    start=False, stop=(tk == NT - 1), skip_group_check=True)
                for ri, m in ((4, 4), (5, 6), (6, 8), (7, 12)):
                    if tk + m < NT:
                        nc.tensor.matmul(
                            out2_all[:, 128 * (tk + m):128 * (tk + m + 1)],
                            Vn[:, tk, :], Pk[:, ri, :],
                            start=False, stop=False, skip_group_check=True)

            bin_ = DR.tile([NC, HD + 1, NB], FP)

            def emit_epilogue(c):
                csl = slice(512 * c, 512 * (c + 1))
                nc.vector.tensor_mul(zX[:, csl], out2_all[:, csl], gT[:, csl])
                for j in (2 * c, 2 * c + 1):
                    nc.sync.dma_start(out=bin_[j],
                                      in_=zX[:, NB * j:NB * (j + 1)])

            for tk in range(NT):
                emit_mm1(tk)
                if tk >= 1:
                    emit_mm2(tk - 1)
                if tk >= 4 and tk % 4 == 0:
                    emit_epilogue(tk // 4 - 1)
            emit_mm2(NT - 1)
            emit_epilogue(3)

            # ---------- AllToAll exchange ----------
            bout_ = DR.tile([NC, HD + 1, NB], FP)
            nc.gpsimd.collective_compute(
                "AllToAll", mybir.AluOpType.bypass,
                replica_groups=[list(range(NC))],
                ins=[bin_[:].opt()], outs=[bout_[:].opt()],
            )

            # ---------- stage 3: normalize + out projection ----------
            # pair-stack heads on partitions: zr2[c, p, n] = z^T[(2p)*64+c] rows
            SRC = NC * (HD + 1) * NB   # element stride between sources in bout_
            bo = bout_[:]
            zr2 = S.tile([128, 4, NB], FP)
            for par in range(2):
                inap = bass.AP(tensor=bo.tensor,
                               offset=bo.offset + par * (HD + 1) * NB,
                               ap=[[NB, 64], [2 * (HD + 1) * NB, 4], [1, NB]])
                nc.sync.dma_start(out=zr2[64 * par:64 * (par + 1), :, :], in_=inap)
            rb2 = S.tile([128, 4, NB], FP)
            for par in range(2):
                inap = bass.AP(tensor=bo.tensor,
                               offset=bo.offset + HD * NB + par * (HD + 1) * NB,
                               ap=[[0, 64], [2 * (HD + 1) * NB, 4], [1, NB]])
                nc.sync.dma_start(out=rb2[64 * par:64 * (par + 1), :, :], in_=inap)
            rr2 = S.tile([128, 4, NB], FP)
            rscr = S.tile([128, 4, NB], FP)
            nc.vector.reciprocal_approx_accurate(rr2[:], rb2[:], rscr[:])
            zn2 = S.tile([128, 4, NB], F16)
            nc.vector.tensor_mul(zn2[:], zr2[:], rr2[:])

            for nt in range(NB // 128):
                nsl = slice(128 * nt, 128 * (nt + 1))
                psY = PS3.tile([128, D], FP, tag="s")
                for p in range(4):
                    nc.tensor.matmul(psY[:], zn2[:, p, nsl], wos[:, p, :],
                                     start=(p == 0), stop=False)
                nc.tensor.matmul(psY[:], ones_r[:, 0:128], bos[:],
                                 start=False, stop=True)
                ysb = W.tile([128, D], FP, tag="ysb")
                nc.scalar.copy(ysb[:], psY[:])
                nc.sync.dma_start(out=yout[nsl, :], in_=ysb[:])

    nc.compile()
    return nc


def _prep_inputs(x, W_qkv, b_qkv, W_out, b_out, W_gate, b_gate,
                 pos_bias, scale_embed, if_gain, disp_amp):
    assert not np.any(np.asarray(scale_embed)), \
        "kernel fast path requires scale_embed == 0"
    xTn = np.ascontiguousarray(np.asarray(x)[0].T.astype(np.float32))  # [D, N]
    W_qkv = np.asarray(W_qkv, dtype=np.float32)
    b_qkv = np.asarray(b_qkv, dtype=np.float32)
    W_gate = np.asarray(W_gate, dtype=np.float32)
    b_gate = np.asarray(b_gate, dtype=np.float32)
    W_out = np.asarray(W_out, dtype=np.float32)
    b_out = np.asarray(b_out, dtype=np.float32)
    pos_bias = np.asarray(pos_bias, dtype=np.float32)
    if_gain = np.asarray(if_gain, dtype=np.float32)
    disp_amp = np.asarray(disp_amp, dtype=np.float32)

    scl = 1.0 / math.sqrt(HD)
    wout16 = np.ascontiguousarray(
        W_out.reshape(4, 128, D).transpose(1, 0, 2)).astype(np.float16)  # [128,4,D]
    xT16 = xTn.astype(np.float16)

    in_maps = []
    for h in range(NC):
        qs = slice(HD * h, HD * (h + 1))
        ks = slice(D + HD * h, D + HD * (h + 1))
        vs = slice(2 * D + HD * h, 2 * D + HD * (h + 1))
        wq = W_qkv[:, qs] * scl
        wk = W_qkv[:, ks]
        wv = W_qkv[:, vs] * if_gain[h]
        wg = W_gate[:, qs]
        bq = b_qkv[qs] * scl
        bk = b_qkv[ks]
        bv = b_qkv[vs] * if_gain[h]
        bg = b_gate[qs]
        eff_pb_h = pos_bias[:, h] + DISP_COS_KERNEL[:, h] * disp_amp[h]
        in_maps.append({
            "xT": xT16,
            "wA": np.ascontiguousarray(
                np.concatenate([wq, wk], axis=1)).astype(np.float16),
            "wB": np.ascontiguousarray(
                np.concatenate([wv, wg], axis=1)).astype(np.float16),
            "bA": np.ascontiguousarray(np.concatenate([bq, bk])),
            "bB": np.ascontiguousarray(np.concatenate([bv, bg])),
            "maskW": _build_masks(eff_pb_h).astype(np.float16),
            "woutS": wout16,
            "bout": b_out.astype(np.float16),
        })
    return in_maps


def kernel(**inputs) -> np.ndarray:
    if "nc" not in _cache:
        _cache["nc"] = _build_module()
    nc = _cache["nc"]
    in_maps = _prep_inputs(**inputs)
    res = run_bass_kernel_spmd(nc, in_maps, core_ids=list(range(NC)))
    y = np.concatenate([res.results[c]["y"] for c in range(NC)], axis=0)
    return y.reshape(B, N, D)


# revision 24
# speedup vs baseline: 2.4607x; 1.0712x over previous
"""Trainium2 Bass kernel for nn_DSQGAttentionQW (sparse offset attention).

Sharding: head-tensor-parallel attention (8 heads -> 8 cores) + AllToAll
re-shard to sequence-parallel for the output projection. Single NEFF launch.
"""
import math
import numpy as np

import concourse.bacc as bacc
import concourse.bass as bass
import concourse.tile as tile
import concourse.mybir as mybir
import concourse.masks as masks
from concourse.bass_utils import run_bass_kernel_spmd

# ---- problem constants (must match reference.py) ----
_DENSE_LOCAL_W = 32
_DYADIC = [48, 64, 96, 128, 192, 256, 384, 512, 768, 1024, 1536, 2048, 3072, 4096]
OFFSETS = np.array(
    sorted(set(range(0, _DENSE_LOCAL_W + 1)) | set(_DYADIC)), dtype=np.int32
)  # [47]
NUM_OFFSETS = len(OFFSETS)
H = 8
_LOG_MAX = math.log(1.0 + 4096.0)
_HEAD_OMEGAS = [0.0, 0.0, 1 * math.pi / _LOG_MAX, 1 * math.pi / _LOG_MAX,
                4 * math.pi / _LOG_MAX, 4 * math.pi / _LOG_MAX,
                6 * math.pi / _LOG_MAX, 6 * math.pi / _LOG_MAX]
_log_d = np.log(1.0 + OFFSETS.astype(np.float64))
DISP_COS_KERNEL = np.zeros((NUM_OFFSETS, H), dtype=np.float32)
for _h, _om in enumerate(_HEAD_OMEGAS):
    if _om > 0.0:
        DISP_COS_KERNEL[:, _h] = np.cos(_om * _log_d)

B, N, D = 1, 2048, 512
HD = D // H
NC = 8
NB = N // NC            # 256: per-core output row block
NT = N // 128           # 16 q-tiles of 128
# Effective k-tile depths m (delta in (128(m-1), 128m]) that can be causal for
# N=2048: depths 16/24/32 (delta >= 2048) are never valid.
R_DEPTHS = [0, 1, 2, 3, 4, 6, 8, 12]
NR = len(R_DEPTHS)

FP = mybir.dt.float32
FR = mybir.dt.float32r
F16 = mybir.dt.float16

_cache = {}


def _build_masks(eff_pb_h: np.ndarray) -> np.ndarray:
    """maskW[ri, kp, i] = exp(eff_pb[offset_idx(delta)]) if delta valid else 0,
    with delta = i - kp + 128*m for depth m = R_DEPTHS[ri]."""
    off_idx = {int(d): i for i, d in enumerate(OFFSETS)}
    kp = np.arange(128)[None, :, None]
    i = np.arange(128)[None, None, :]
    m = np.array(R_DEPTHS)[:, None, None]
    delta = i - kp + 128 * m  # [NR, 128, 128]
    w = np.zeros((NR, 128, 128), dtype=np.float32)
    for d, oi in off_idx.items():
        sel = delta == d
        if sel.any():
            w[sel] = math.exp(float(eff_pb_h[oi]))
    return w


def _r(ap):
    return ap.bitcast(FR)


def _build_module():
    nc = bacc.Bacc("TRN2", target_bir_lowering=False, debug=False, num_devices=NC)

    xT = nc.dram_tensor("xT", [D, N], F16, kind="ExternalInput").ap()
    wA = nc.dram_tensor("wA", [D, 128], F16, kind="ExternalInput").ap()   # [Wq|Wk]
    wB = nc.dram_tensor("wB", [D, 128], F16, kind="ExternalInput").ap()   # [Wv|Wg]
    bA = nc.dram_tensor("bA", [128], FP, kind="ExternalInput").ap()
    bB = nc.dram_tensor("bB", [128], FP, kind="ExternalInput").ap()
    maskW = nc.dram_tensor("maskW", [NR, 128, 128], F16, kind="ExternalInput").ap()
    woutS = nc.dram_tensor("woutS", [128, 4, D], F16, kind="ExternalInput").ap()
    bout = nc.dram_tensor("bout", [D], F16, kind="ExternalInput").ap()
    yout = nc.dram_tensor("y", [NB, D], FP, kind="ExternalOutput").ap()

    with tile.TileContext(nc) as tc:
        with (
            tc.tile_pool(name="singles", bufs=1) as S,
            tc.tile_pool(name="work", bufs=3) as W,
            tc.tile_pool(name="pk", bufs=3) as PK,
            tc.tile_pool(name="ps", bufs=1, space="PSUM") as PS,
            tc.tile_pool(name="ps3", bufs=3, space="PSUM") as PS3,
            tc.tile_pool(name="pso", bufs=1, space="PSUM") as PSO,
            tc.tile_pool(name="dram", bufs=1, space="DRAM") as DR,
        ):
            # ---------- PE warm-up (HAM) during initial DMA window ----------
            wconst = S.tile([128, 512], F16)
            nc.vector.memset(wconst[:], 1.0)
            psW = PS3.tile([128, 512], FP, tag="s")
            for _ in range(16):
                nc.tensor.matmul(psW[:], wconst[:, 0:128], wconst[:],
                                 start=True, stop=True)

            # ---------- constants / loads ----------
            ident = S.tile([128, 128], F16)
            masks.make_identity(nc, ident[:])
            ones_r = S.tile([1, 128], F16)
            nc.vector.memset(ones_r[:], 1.0)

            wAs = S.tile([128, 4, 128], F16)
            nc.sync.dma_start(out=wAs[:], in_=wA.rearrange("(ct p) o -> p ct o", p=128))
            wBs = S.tile([128, 4, 128], F16)
            nc.sync.dma_start(out=wBs[:], in_=wB.rearrange("(ct p) o -> p ct o", p=128))
            bAs = S.tile([128, 1], FP)
            nc.sync.dma_start(out=bAs[:], in_=bA[:, None])
            bBs = S.tile([128, 1], FP)
            nc.sync.dma_start(out=bBs[:], in_=bB[:, None])

            xs = S.tile([128, 4, N], F16)
            xT_r = xT.rearrange("(ct p) n -> p ct n", p=128)
            for nch in range(4):
                for ct in range(4):
                    nsl = slice(512 * nch, 512 * (nch + 1))
                    nc.sync.dma_start(out=xs[:, ct, nsl], in_=xT_r[:, ct, nsl])

            mws = S.tile([128, NR, 128], F16)
            nc.sync.dma_start(out=mws[:], in_=maskW.rearrange("r kp i -> kp r i"))
            wos = S.tile([128, 4, D], F16)
            nc.sync.dma_start(out=wos[:], in_=woutS[:])
            bos = S.tile([1, D], F16)
            nc.sync.dma_start(out=bos[:], in_=bout[None, :])

            # ---------- MM-A: qT / kT / vT / gT (fp32r, N=512) ----------
            qT = S.tile([64, N], F16)       # pre-scaled by 1/sqrt(HD)
            kT = S.tile([64, N], F16)
            vT = S.tile([64, N], F16)       # if_gain folded
            gT = S.tile([HD + 1, N], FP)   # sigmoid gate; row 64 = 1.0 (denom)
            nc.vector.memset(gT[HD:HD + 1, :], 1.0)
            Vn = S.tile([128, NT, HD + 1], F16)
            nc.vector.memset(Vn[:, :, HD:HD + 1], 1.0)

            def emit_transpose(t):
                psT = PS3.tile([128, 64], F16, tag="s")
                nc.tensor.transpose(psT[:], vT[:, 128 * t:128 * (t + 1)],
                                    ident[0:64, 0:64])
                nc.vector.tensor_copy(Vn[:, t, 0:HD], psT[:])

            for nch in range(4):
                nsl = slice(512 * nch, 512 * (nch + 1))
                psA = PS.tile([128, 512], FP, tag="mma")
                psB = PS3.tile([128, 512], FP, tag="s")
                for ct in range(4):
                    nc.tensor.matmul(psA[:], wAs[:, ct, :], xs[:, ct, nsl],
                                     start=(ct == 0), stop=(ct == 3))
                for ct in range(4):
                    nc.tensor.matmul(psB[:], wBs[:, ct, :], xs[:, ct, nsl],
                                     start=(ct == 0), stop=(ct == 3))
                # biased copies: q,k,v on DVE; gate sigmoid on ACT
                nc.vector.tensor_scalar_add(qT[:, nsl], psA[0:64, :], bAs[0:64])
                nc.vector.tensor_scalar_add(kT[:, nsl], psA[64:128, :], bAs[64:128])
                nc.scalar.activation(vT[:, nsl], psB[0:64, :],
                                     mybir.ActivationFunctionType.Identity,
                                     bias=bBs[0:64], scale=1.0)
                nc.scalar.activation(gT[0:HD, nsl], psB[64:128, :],
                                     mybir.ActivationFunctionType.Sigmoid,
                                     bias=bBs[64:128], scale=1.0)
                if nch >= 1:
                    for t in range(4 * (nch - 1), 4 * nch):
                        emit_transpose(t)


            for t in range(12, 16):
                emit_transpose(t)

            # ---------- attention (k-tile-major, fp16) ----------
            # out2_all: [65, N] PSUM accumulator (4 banks). Zero-init via K=1
            # start=True matmuls so subsequent MMs accumulate via has_written.
            out2_all = PSO.tile([HD + 1, N], FP)
            zrow = S.tile([1, 512], F16)
            nc.vector.memset(zrow[:], 0.0)
            z65 = S.tile([1, HD + 1], F16)
            nc.vector.memset(z65[:], 0.0)
            for c in range(4):
                nc.tensor.matmul(out2_all[:, 512 * c:512 * (c + 1)], z65[:],
                                 zrow[:], start=True, stop=False)

            zX = S.tile([HD + 1, N], F16)  # gated attn out^T + denom row

            Pks = [None] * NT

            def emit_mm1(tk):
                ksl = slice(128 * tk, 128 * (tk + 1))
                groups = []
                nq_near = min(3, NT - tk)
                groups.append(([0, 1, 2][:nq_near], tk, nq_near))
                nq34 = max(0, min(2, NT - tk - 3))
                if nq34:
                    groups.append(([3, 4][:nq34], tk + 3, nq34))
                if tk + 8 < NT:
                    groups.append(([6, None, 8], tk + 6, 3))
                elif tk + 6 < NT:
                    groups.append(([6], tk + 6, 1))
                if tk + 12 < NT:
                    groups.append(([12], tk + 12, 1))
                Pk = PK.tile([128, NR, 128], F16, tag="Pk")
                Pks[tk] = Pk
                for ms, q_lo, nq in groups:
                    qsl = slice(128 * q_lo, 128 * (q_lo + nq))
                    psS = PS3.tile([128, 384], FP, tag="s")
                    nc.tensor.matmul(psS[:, 0:128 * nq], kT[:, ksl],
                                     qT[:, qsl], start=True, stop=True)
                    expS = W.tile([128, 384], F16, tag="expS")
                    nc.scalar.activation(expS[:, 0:128 * nq], psS[:, 0:128 * nq],
                                         mybir.ActivationFunctionType.Exp)
                    ris = [R_DEPTHS.index(m) for m in ms if m is not None]
                    if ms == [6, None, 8]:
                        b = expS[:]
                        srcap = bass.AP(tensor=b.tensor, offset=b.offset,
                                        ap=[b.ap[0], [256, 2], [1, 128]])
                        nc.gpsimd.tensor_mul(Pk[:, ris[0]:ris[0] + 2, :],
                                             srcap, mws[:, ris[0]:ris[0] + 2, :])
                    elif ris[0] >= 5:
                        nc.gpsimd.tensor_mul(
                            Pk[:, ris[0]:ris[0] + len(ris), :],
                            expS[:, 0:128 * len(ris)],
                            mws[:, ris[0]:ris[0] + len(ris), :])
                    else:
                        nc.vector.tensor_mul(
                            Pk[:, ris[0]:ris[0] + len(ris), :],
                            expS[:, 0:128 * len(ris)],
                            mws[:, ris[0]:ris[0] + len(ris), :])

            def emit_mm2(tk):
                Pk = Pks[tk]
                nq1 = min(4, NT - tk)
                nc.tensor.matmul(
                    out2_all[:, 128 * tk:128 * (tk + nq1)],
                    Vn[:, tk, :], Pk[:, 0:nq1, :],
                    start=False, stop=(tk == NT - 1), skip_group_check=True)
                for ri, m in ((4, 4), (5, 6), (6, 8), (7, 12)):
                    if tk + m < NT:
                        nc.tensor.matmul(
                            out2_all[:, 128 * (tk + m):128 * (tk + m + 1)],
                            Vn[:, tk, :], Pk[:, ri, :],
                            start=False, stop=False, skip_group_check=True)

            bin_ = DR.tile([NC, HD + 1, NB], F16)

            def emit_epilogue(c):
                csl = slice(512 * c, 512 * (c + 1))
                nc.vector.tensor_mul(zX[:, csl], out2_all[:, csl], gT[:, csl])
                for j in (2 * c, 2 * c + 1):
                    nc.sync.dma_start(out=bin_[j],
                                      in_=zX[:, NB * j:NB * (j + 1)])

            for tk in range(NT):
                emit_mm1(tk)
                if tk >= 1:
                    emit_mm2(tk - 1)
                if tk >= 4 and tk % 4 == 0:
                    emit_epilogue(tk // 4 - 1)
            emit_mm2(NT - 1)
            emit_epilogue(3)

            # keep PE warm through the collective: dummy MMs gated on zX
            dzz = W.tile([HD + 1, 512], F16, tag="ysb")
            nc.vector.tensor_copy(dzz[:], zX[:, 0:512])
            psW2 = PS3.tile([128, 512], FP, tag="s")
            for _ in range(40):
                nc.tensor.matmul(psW2[:], dzz[:, 0:128], dzz[:],
                                 start=True, stop=True)

            # ---------- AllToAll exchange ----------
            bout_ = DR.tile([NC, HD + 1, NB], F16)
            nc.gpsimd.collective_compute(
                "AllToAll", mybir.AluOpType.bypass,
                replica_groups=[list(range(NC))],
                ins=[bin_[:].opt()], outs=[bout_[:].opt()],
            )

            # ---------- stage 3: normalize + out projection ----------
            # pair-stack heads on partitions: zr2[c, p, n] = z^T[(2p)*64+c] rows
            SRC = NC * (HD + 1) * NB   # element stride between sources in bout_
            bo = bout_[:]
            zr2 = S.tile([128, 4, NB], F16)
            for par in range(2):
                inap = bass.AP(tensor=bo.tensor,
                               offset=bo.offset + par * (HD + 1) * NB,
                               ap=[[NB, 64], [2 * (HD + 1) * NB, 4], [1, NB]])
                nc.sync.dma_start(out=zr2[64 * par:64 * (par + 1), :, :], in_=inap)
            rb2 = S.tile([128, 4, NB], F16)
            for par in range(2):
                inap = bass.AP(tensor=bo.tensor,
                               offset=bo.offset + HD * NB + par * (HD + 1) * NB,
                               ap=[[0, 64], [2 * (HD + 1) * NB, 4], [1, NB]])
                nc.sync.dma_start(out=rb2[64 * par:64 * (par + 1), :, :], in_=inap)
            rbf = S.tile([128, 4, NB], FP)
            nc.vector.tensor_copy(rbf[:], rb2[:])
            rr2 = S.tile([128, 4, NB], FP)
            rscr = S.tile([128, 4, NB], FP)
            nc.vector.reciprocal_approx_accurate(rr2[:], rbf[:], rscr[:])
            zn2 = S.tile([128, 4, NB], F16)
            nc.vector.tensor_mul(zn2[:], zr2[:], rr2[:])

            for nt in range(NB // 128):
                nsl = slice(128 * nt, 128 * (nt + 1))
                psY = PS3.tile([128, D], FP, tag="s")
                for p in range(4):
                    nc.tensor.matmul(psY[:], zn2[:, p, nsl], wos[:, p, :],
                                     start=(p == 0), stop=False)
                nc.tensor.matmul(psY[:], ones_r[:, 0:128], bos[:],
                                 start=False, stop=True)
                ysb = W.tile([128, D], FP, tag="ysb")
                nc.scalar.copy(ysb[:], psY[:])
                nc.sync.dma_start(out=yout[nsl, :], in_=ysb[:])

    nc.compile()
    return nc


def _prep_inputs(x, W_qkv, b_qkv, W_out, b_out, W_gate, b_gate,
                 pos_bias, scale_embed, if_gain, disp_amp):
    assert not np.any(np.asarray(scale_embed)), \
        "kernel fast path requires scale_embed == 0"
    xTn = np.ascontiguousarray(np.asarray(x)[0].T.astype(np.float32))  # [D, N]
    W_qkv = np.asarray(W_qkv, dtype=np.float32)
    b_qkv = np.asarray(b_qkv, dtype=np.float32)
    W_gate = np.asarray(W_gate, dtype=np.float32)
    b_gate = np.asarray(b_gate, dtype=np.float32)
    W_out = np.asarray(W_out, dtype=np.float32)
    b_out = np.asarray(b_out, dtype=np.float32)
    pos_bias = np.asarray(pos_bias, dtype=np.float32)
    if_gain = np.asarray(if_gain, dtype=np.float32)
    disp_amp = np.asarray(disp_amp, dtype=np.float32)

    scl = 1.0 / math.sqrt(HD)
    wout16 = np.ascontiguousarray(
        W_out.reshape(4, 128, D).transpose(1, 0, 2)).astype(np.float16)  # [128,4,D]
    xT16 = xTn.astype(np.float16)

    in_maps = []
    for h in range(NC):
        qs = slice(HD * h, HD * (h + 1))
        ks = slice(D + HD * h, D + HD * (h + 1))
        vs = slice(2 * D + HD * h, 2 * D + HD * (h + 1))
        wq = W_qkv[:, qs] * scl
        wk = W_qkv[:, ks]
        wv = W_qkv[:, vs] * if_gain[h]
        wg = W_gate[:, qs]
        bq = b_qkv[qs] * scl
        bk = b_qkv[ks]
        bv = b_qkv[vs] * if_gain[h]
        bg = b_gate[qs]
        eff_pb_h = pos_bias[:, h] + DISP_COS_KERNEL[:, h] * disp_amp[h]
        in_maps.append({
            "xT": xT16,
            "wA": np.ascontiguousarray(
                np.concatenate([wq, wk], axis=1)).astype(np.float16),
            "wB": np.ascontiguousarray(
                np.concatenate([wv, wg], axis=1)).astype(np.float16),
            "bA": np.ascontiguousarray(np.concatenate([bq, bk])),
            "bB": np.ascontiguousarray(np.concatenate([bv, bg])),
            "maskW": _build_masks(eff_pb_h).astype(np.float16),
            "woutS": wout16,
            "bout": b_out.astype(np.float16),
        })
    return in_maps


def kernel(**inputs) -> np.ndarray:
    if "nc" not in _cache:
        _cache["nc"] = _build_module()
    nc = _cache["nc"]
    in_maps = _prep_inputs(**inputs)
    res = run_bass_kernel_spmd(nc, in_maps, core_ids=list(range(NC)))
    y = np.concatenate([res.results[c]["y"] for c in range(NC)], axis=0)
    return y.reshape(B, N, D)
